# revision 11
# baseline (speedup 1.0000x reference)
"""Trainium2 Bass kernel for top-1 MoE (nn_MoE_48808008352179).

Expert parallelism across 8 NeuronCores: core e owns expert e's weights.
Full hidden_states is replicated to every core's HBM; each core computes
top-1 routing on-device (logits -> softmax/argmax -> capacity-limited
slot assignment via triangular-matmul cumsum), compacts its own expert's
token list via indirect-DMA scatter, gathers those token rows, runs the
expert MLP (two fp32 GEMMs + tanh-approx gelu) on the tensor engine, and
writes gate-scaled output rows. The host scatters each expert's rows back
to token positions (the "combine"), which is exact because top-1 routing
makes expert outputs disjoint over tokens.

kernel(**inputs) takes the full unsharded inputs and returns
(out[B,S,D] f32, l_aux f32 scalar, exp_counts[E] int32) matching the
reference.
"""

import numpy as np

# Problem dimensions (hardcoded per contest rules).
B, S, D, E, DFF = 2, 2048, 1024, 8, 4096
T = B * S                      # 4096 tokens
C = max(4, -(-T // E))         # 512 capacity (CAP_FACTOR=1.0, MIN_CAP=4)
P = 128                        # partitions
NT = T // P                    # 32 token tiles
DK = D // P                    # 8 contraction tiles over D
FK = DFF // P                  # 32 contraction tiles over DFF
N_CORES = 8

_SLOT_BIG = 65536.0            # sentinel slot for dropped tokens (> C-1 -> skipped)


def emit_moe(tc, outs, ins):
    """Emit the per-core MoE program into TileContext tc.

    ins:  x[T,D], wgr[128,DK*E], w1[D,DFF], b1c[128,FK], w2[DFF,D],
          b2rep[128,D], selrep[128,E], ident[128,128], u128[128,128],
          u32s[32,32], onesp[128,1], selcol[8,1]
    outs: rows[C,D] f32, toks[C,1] i32, gatesl[C,1] f32,
          counts[1,E] i32, laux[1,1] f32
    """
    import concourse.mybir as mybir
    import concourse.bass as bass
    from concourse.tile_rust import add_dep_helper

    nc = tc.nc
    f32 = mybir.dt.float32
    i32 = mybir.dt.int32
    u32 = mybir.dt.uint32
    f32r = mybir.dt.float32r
    AX = mybir.AxisListType
    OP = mybir.AluOpType
    ACTF = mybir.ActivationFunctionType

    x_d = ins["x"]
    wgr_d = ins["wgr"]
    w1_d = ins["w1"]
    b1c_d = ins["b1c"]
    w2_d = ins["w2"]
    b2rep_d = ins["b2rep"]
    selrep_d = ins["selrep"]
    ident_d = ins["ident"]
    u128_d = ins["u128"]
    u32s_d = ins["u32s"]
    onesp_d = ins["onesp"]
    selcol_d = ins["selcol"]

    rows_d = outs["rows"]
    toks2_d = outs["toks2"]
    counts_d = outs["counts"]
    laux_d = outs["laux"]

    from contextlib import ExitStack

    ctx = ExitStack()
    with ctx:
        cst = ctx.enter_context(tc.tile_pool(name="cst", bufs=1))
        per = ctx.enter_context(tc.tile_pool(name="per", bufs=1))
        rt = ctx.enter_context(tc.tile_pool(name="rt", bufs=2))
        dsp = ctx.enter_context(tc.tile_pool(name="dsp", bufs=2))
        w1p = ctx.enter_context(tc.tile_pool(name="w1p", bufs=2))
        w2p = ctx.enter_context(tc.tile_pool(name="w2p", bufs=4))
        outp = ctx.enter_context(tc.tile_pool(name="outp", bufs=3))

        # ---- constants from host ----
        ident = cst.tile([P, P], f32)
        nc.sync.dma_start(out=ident[:], in_=ident_d[:])
        u128 = cst.tile([P, P], f32)
        nc.sync.dma_start(out=u128[:], in_=u128_d[:])
        u32s = cst.tile([32, 32], f32)
        nc.sync.dma_start(out=u32s[:], in_=u32s_d[:])
        wgr = cst.tile([P, DK * E], f32)
        nc.sync.dma_start(out=wgr[:], in_=wgr_d[:])
        selrep = cst.tile([P, E], f32)
        nc.sync.dma_start(out=selrep[:], in_=selrep_d[:])
        sel32 = cst.tile([P, 4 * E], f32)
        for _j in range(4):
            nc.vector.tensor_copy(sel32[:, _j * E : (_j + 1) * E], selrep[:])
        bigslot4 = cst.tile([P, 4], f32)
        nc.vector.memset(bigslot4[:], _SLOT_BIG)
        onesp = cst.tile([P, 1], f32)
        nc.sync.dma_start(out=onesp[:], in_=onesp_d[:])
        selcol = cst.tile([E, 1], f32)
        nc.sync.dma_start(out=selcol[:], in_=selcol_d[:])
        b1c = cst.tile([P, FK], f32)
        nc.sync.dma_start(out=b1c[:], in_=b1c_d[:])
        b2rep = cst.tile([P, D], f32)
        nc.sync.dma_start(out=b2rep[:], in_=b2rep_d[:])

        iota8i = cst.tile([P, E], i32)
        nc.gpsimd.iota(iota8i[:], pattern=[[1, E]], base=0, channel_multiplier=0)
        iota8 = cst.tile([P, E], f32)
        nc.vector.tensor_copy(iota8[:], iota8i[:])
        big8 = cst.tile([P, E], f32)
        nc.vector.memset(big8[:], float(E))
        bigslot = cst.tile([P, 1], f32)
        nc.vector.memset(bigslot[:], _SLOT_BIG)

        # zero-init the slot table (CoreSim poisons output DRAM; HW pre-zeros)
        zinit = cst.tile([P, 2 * C // P], i32)
        nc.vector.memset(zinit[:], 0)
        zi_t = nc.sync.dma_start(
            out=toks2_d.rearrange("(c p) two -> p c two", p=P), in_=zinit[:]
        )

        # ---- persistent state ----
        mask_all = per.tile([P, NT * E], f32)
        gate_all = per.tile([P, NT], f32)
        xte = per.tile([P, DK, C], f32r)      # dispatched tokens, transposed
        ht = per.tile([P, FK, C], f32r)       # gelu(x @ w1 + b1), transposed

        scatter_insts = []

        # ================= Phase A: logits + per-tile routing =================
        with (
            tc.tile_pool(name="psMe", bufs=1, space="PSUM") as psMe,
            tc.tile_pool(name="psTot", bufs=1, space="PSUM") as psTot,
        ):
            me_ps = psMe.tile([1, E], f32, space="PSUM")
            totT_ps = psTot.tile([E, NT], f32, space="PSUM")

            phaseA = tc.tile_pool(name="psA", bufs=2, space="PSUM")
            psA = phaseA.__enter__()
            phaseL = tc.tile_pool(name="psL", bufs=2, space="PSUM")
            psL = phaseL.__enter__()
            for i in range(NT):
                x_sb = rt.tile([P, D], f32, tag="x_sb")
                nc.sync.dma_start(out=x_sb[:], in_=x_d[i * P : (i + 1) * P, :])

                # transpose x tile -> xT chunks [d, t], batched 4 per psum bank
                xT_sb = rt.tile([P, D], f32, tag="xT_sb")
                for g in range(2):
                    tp = psA.tile([P, 512], f32, space="PSUM", tag="tpA")
                    for j in range(4):
                        k = g * 4 + j
                        nc.tensor.transpose(
                            out=tp[:, j * P : (j + 1) * P],
                            in_=x_sb[:, k * P : (k + 1) * P],
                            identity=ident[:],
                        )
                    if g == 0:
                        nc.scalar.copy(xT_sb[:, g * 512 : (g + 1) * 512], tp[:])
                    else:
                        nc.vector.tensor_copy(
                            xT_sb[:, g * 512 : (g + 1) * 512], tp[:]
                        )

                # logitsT [E, 128] = wg^T @ xT  (wg stationary)
                lgT_ps = psL.tile([E, P], f32, space="PSUM", tag="lgT")
                for k in range(DK):
                    nc.tensor.matmul(
                        lgT_ps[:],
                        lhsT=wgr[:, k * E : (k + 1) * E],
                        rhs=xT_sb[:, k * P : (k + 1) * P],
                        start=(k == 0),
                        stop=(k == DK - 1),
                    )
                lgT_sb = rt.tile([E, P], f32, tag="lgT_sb")
                nc.vector.tensor_copy(lgT_sb[:], lgT_ps[:])
                lg_ps = psL.tile([P, E], f32, space="PSUM", tag="lg")
                nc.tensor.transpose(
                    out=lg_ps[:], in_=lgT_sb[:], identity=ident[:E, :E]
                )
                lg = rt.tile([P, E], f32, tag="lg_sb")
                nc.vector.tensor_copy(lg[:], lg_ps[:])

                # routing math on [128, 8]
                m = rt.tile([P, 1], f32, tag="m")
                nc.vector.tensor_reduce(m[:], lg[:], axis=AX.X, op=OP.max)
                neg_m = rt.tile([P, 1], f32, tag="neg_m")
                nc.scalar.mul(neg_m[:], m[:], -1.0)
                exps = rt.tile([P, E], f32, tag="exps")
                ssum = rt.tile([P, 1], f32, tag="ssum")
                nc.scalar.activation(
                    exps[:], lg[:], ACTF.Exp, bias=neg_m[:], accum_out=ssum[:]
                )
                nc.vector.reciprocal(gate_all[:, i : i + 1], ssum[:])

                eq = rt.tile([P, E], u32, tag="eq")
                nc.vector.tensor_tensor(
                    out=eq[:], in0=lg[:], in1=m[:].to_broadcast([P, E]),
                    op=OP.is_equal,
                )
                masked = rt.tile([P, E], f32, tag="masked")
                nc.vector.select(masked[:], eq[:], iota8[:], big8[:])
                idxf = rt.tile([P, 1], f32, tag="idxf")
                nc.vector.tensor_reduce(idxf[:], masked[:], axis=AX.X, op=OP.min)
                mask_i = mask_all[:, i * E : (i + 1) * E]
                nc.vector.tensor_tensor(
                    out=mask_i, in0=iota8[:], in1=idxf[:].to_broadcast([P, E]),
                    op=OP.is_equal,
                )

                # l_aux numerator: accumulate sum_t softmax probs per expert
                probs = rt.tile([P, E], f32, tag="probs")
                nc.vector.tensor_tensor(
                    out=probs[:], in0=exps[:],
                    in1=gate_all[:, i : i + 1].to_broadcast([P, E]), op=OP.mult,
                )
                nc.tensor.matmul(
                    me_ps[:], lhsT=onesp[:], rhs=probs[:],
                    start=(i == 0), stop=(i == NT - 1), skip_group_check=True,
                )
                # per-tile expert totals into column i of totT [E, NT]
                nc.tensor.matmul(
                    totT_ps[:, i : i + 1], lhsT=mask_i, rhs=onesp[:],
                    start=True, stop=True, skip_group_check=True,
                )

            phaseL.__exit__(None, None, None)
            phaseA.__exit__(None, None, None)

            # ============== Phase B: tile offsets, counts, l_aux ==============
            totT_sb = rt.tile([E, NT], f32)
            nc.vector.tensor_copy(totT_sb[:], totT_ps[:])

            with tc.tile_pool(name="psLoc", bufs=2, space="PSUM") as psLoc:
                tot_ps2 = psLoc.tile([NT, E], f32, space="PSUM", tag="smallB")
                nc.tensor.transpose(
                    out=tot_ps2[:], in_=totT_sb[:], identity=ident[:E, :E]
                )
                tot_sb = rt.tile([NT, E], f32)
                nc.vector.tensor_copy(tot_sb[:], tot_ps2[:])

                offs_ps = psLoc.tile([32, E], f32, space="PSUM", tag="smallB")
                nc.tensor.matmul(
                    offs_ps[:], lhsT=u32s[:], rhs=tot_sb[:], start=True, stop=True
                )
                offs_sb = rt.tile([32, E], f32)
                nc.vector.tensor_copy(offs_sb[:], offs_ps[:])

                # own expert's per-tile offsets, replicated to all partitions
                offsT_ps = psLoc.tile([E, NT], f32, space="PSUM", tag="smallB")
                nc.tensor.transpose(
                    out=offsT_ps[:], in_=offs_sb[:], identity=ident[:NT, :NT]
                )
                offsT_sb = rt.tile([E, NT], f32)
                nc.vector.tensor_copy(offsT_sb[:], offsT_ps[:])
                ownoff_ps = psLoc.tile([1, NT], f32, space="PSUM", tag="smallB")
                nc.tensor.matmul(
                    ownoff_ps[:], lhsT=selcol[:], rhs=offsT_sb[:],
                    start=True, stop=True,
                )
                ownoff_sb = rt.tile([1, NT], f32)
                nc.vector.tensor_copy(ownoff_sb[:], ownoff_ps[:])
                offs_bc = per.tile([P, NT], f32)
                nc.gpsimd.partition_broadcast(offs_bc[:], ownoff_sb[:])

                cnt_ps = psLoc.tile([1, E], f32, space="PSUM", tag="smallB")
                nc.tensor.matmul(
                    cnt_ps[:], lhsT=onesp[:32, :], rhs=tot_sb[:],
                    start=True, stop=True,
                )
                cnt_sb = rt.tile([1, E], f32)
                nc.vector.tensor_copy(cnt_sb[:], cnt_ps[:])
                cnt_i = rt.tile([1, E], i32)
                nc.vector.tensor_copy(cnt_i[:], cnt_sb[:])
                nc.sync.dma_start(out=counts_d[:], in_=cnt_i[:])

                me_sb = rt.tile([1, E], f32)
                nc.vector.tensor_copy(me_sb[:], me_ps[:])
                lx = rt.tile([1, E], f32)
                nc.vector.tensor_tensor(
                    out=lx[:], in0=me_sb[:], in1=cnt_sb[:], op=OP.mult
                )
                lx1 = rt.tile([1, 1], f32)
                nc.vector.tensor_reduce(lx1[:], lx[:], axis=AX.X, op=OP.add)
                laux_sb = rt.tile([1, 1], f32)
                nc.scalar.mul(laux_sb[:], lx1[:], float(E) / float(T) / float(T))
                nc.sync.dma_start(out=laux_d[:], in_=laux_sb[:])

                # ============ Phase C: slots + compaction scatters ============
                for i4 in range(NT // 4):
                    msl = mask_all[:, i4 * 4 * E : (i4 + 1) * 4 * E]
                    loc_ps = psLoc.tile([P, 4 * E], f32, space="PSUM", tag="loc")
                    nc.tensor.matmul(
                        loc_ps[:], lhsT=u128[:], rhs=msl, start=True, stop=True
                    )
                    ownm = rt.tile([P, 4 * E], f32, tag="ownm")
                    nc.vector.tensor_tensor(
                        out=ownm[:], in0=msl, in1=sel32[:], op=OP.mult
                    )
                    own1 = rt.tile([P, 4], f32, tag="own1")
                    nc.vector.tensor_reduce(
                        own1[:], ownm[:].rearrange("p (f e) -> p f e", e=E),
                        axis=AX.X, op=OP.add,
                    )
                    lm = rt.tile([P, 4 * E], f32, tag="lm")
                    nc.vector.tensor_tensor(
                        out=lm[:], in0=loc_ps[:], in1=ownm[:], op=OP.mult
                    )
                    own_loc = rt.tile([P, 4], f32, tag="own_loc")
                    nc.vector.tensor_reduce(
                        own_loc[:], lm[:].rearrange("p (f e) -> p f e", e=E),
                        axis=AX.X, op=OP.add,
                    )
                    # global 0-based slot = tile cumsum + tile offset - 1
                    nc.vector.tensor_tensor(
                        out=own_loc[:], in0=own_loc[:],
                        in1=offs_bc[:, i4 * 4 : (i4 + 1) * 4], op=OP.add,
                    )
                    nc.vector.tensor_scalar(
                        out=own_loc[:], in0=own_loc[:], scalar1=1.0, scalar2=None,
                        op0=OP.subtract,
                    )
                    cmp = rt.tile([P, 4], f32, tag="cmp")
                    nc.vector.tensor_scalar(
                        out=cmp[:], in0=own_loc[:], scalar1=float(C), scalar2=None,
                        op0=OP.is_lt,
                    )
                    keep = rt.tile([P, 4], u32, tag="keep")
                    nc.vector.tensor_tensor(
                        out=keep[:], in0=cmp[:], in1=own1[:], op=OP.mult
                    )
                    slotf = rt.tile([P, 4], f32, tag="slotf")
                    nc.vector.select(slotf[:], keep[:], own_loc[:], bigslot4[:])
                    sloti = rt.tile([P, 4], i32, tag="sloti")
                    nc.vector.tensor_copy(sloti[:], slotf[:])

                    for j in range(4):
                        i = i4 * 4 + j
                        packed = rt.tile([P, 2], i32, tag="packed")
                        nc.gpsimd.iota(
                            packed[:, 0:1], pattern=[[1, 1]], base=i * P,
                            channel_multiplier=1,
                        )
                        nc.vector.tensor_copy(
                            packed[:, 1:2], gate_all[:, i : i + 1].bitcast(i32)
                        )
                        s1 = nc.gpsimd.indirect_dma_start(
                            out=toks2_d[:],
                            out_offset=bass.IndirectOffsetOnAxis(
                                ap=sloti[:, j : j + 1], axis=0),
                            in_=packed[:], in_offset=None,
                            bounds_check=C - 1, oob_is_err=False,
                        )
                        add_dep_helper(s1.ins, zi_t.ins, True, "toks2 zinit WAW")
                        scatter_insts.append(s1)

        # ================= Phase D: dispatch gather + transpose ===============
        gate_sl = per.tile([P, C // P], f32)
        tok_sl = per.tile([P, C // P], i32)
        with tc.tile_pool(name="psTr", bufs=4, space="PSUM") as psTr:
            for c in range(C // P):
                rd_t = nc.sync.dma_start(
                    out=tok_sl[:, c : c + 1], in_=toks2_d[c * P : (c + 1) * P, 0:1]
                )
                rd_g = nc.sync.dma_start(
                    out=gate_sl[:, c : c + 1],
                    in_=toks2_d[c * P : (c + 1) * P, 1:2].bitcast(f32),
                )
                for s in scatter_insts:
                    add_dep_helper(rd_t.ins, s.ins, True, "toks RAW")
                    add_dep_helper(rd_g.ins, s.ins, True, "gates RAW")

                xg = dsp.tile([P, D], f32, tag="xg")
                nc.gpsimd.indirect_dma_start(
                    out=xg[:], out_offset=None,
                    in_=x_d[:],
                    in_offset=bass.IndirectOffsetOnAxis(ap=tok_sl[:, c : c + 1], axis=0),
                    bounds_check=T - 1, oob_is_err=False,
                )
                for k in range(DK):
                    tp = psTr.tile([P, P], f32, space="PSUM", tag="tpD")
                    nc.tensor.transpose(
                        out=tp[:], in_=xg[:, k * P : (k + 1) * P], identity=ident[:]
                    )
                    nc.vector.tensor_copy(
                        xte[:, k, c * P : (c + 1) * P], tp[:]
                    )

        # ================= Phase E: GEMM1 + gelu -> hT ========================
        with tc.tile_pool(name="psH", bufs=3, space="PSUM") as psH:
            for g in range(8):          # groups of 4 DFF tiles
                w1g = w1p.tile([P, DK, 512], f32r, tag="w1g")
                for k in range(DK):
                    nc.sync.dma_start(
                        out=w1g[:, k, :],
                        in_=w1_d[k * P : (k + 1) * P, g * 512 : (g + 1) * 512].bitcast(f32r),
                    )
                for mm in range(4):
                    mt = g * 4 + mm
                    h_ps = psH.tile([P, C], f32, space="PSUM", tag="h")
                    for k in range(DK):
                        nc.tensor.matmul(
                            h_ps[:],
                            lhsT=w1g[:, k, mm * P : (mm + 1) * P],
                            rhs=xte[:, k, :],
                            start=(k == 0),
                            stop=(k == DK - 1),
                        )
                    nc.scalar.activation(
                        ht[:, mt, :], h_ps[:], ACTF.Gelu_apprx_tanh,
                        bias=b1c[:, mt : mt + 1],
                    )

        # ================= Phase F: GEMM2 + bias + gate scale =================
        with tc.tile_pool(name="psO", bufs=1, space="PSUM") as psO:
            o_ps = []
            for ct in range(4):
                row = []
                for n in range(2):
                    o_tile = psO.tile([P, 512], f32, space="PSUM", tag=f"o{ct}{n}",
                                      name=f"o_ps_{ct}_{n}")
                    row.append(o_tile)
                o_ps.append(row)
            for k in range(FK):
                w2s = w2p.tile([P, D], f32r, tag="w2s")
                nc.sync.dma_start(
                    out=w2s[:], in_=w2_d[k * P : (k + 1) * P, :].bitcast(f32r))
                for ct in range(4):
                    for n in range(2):
                        nc.tensor.matmul(
                            o_ps[ct][n][:],
                            lhsT=ht[:, k, ct * P : (ct + 1) * P],
                            rhs=w2s[:, n * 512 : (n + 1) * 512],
                            start=(k == 0),
                            stop=(k == FK - 1),
                        )
            for ct in range(4):
                for n in range(2):
                    ot = outp.tile([P, 512], f32, tag="ot")
                    nc.vector.tensor_tensor(
                        out=ot[:], in0=o_ps[ct][n][:],
                        in1=b2rep[:, n * 512 : (n + 1) * 512], op=OP.add,
                    )
                    nc.scalar.mul(ot[:], ot[:], gate_sl[:, ct : ct + 1])
                    nc.sync.dma_start(
                        out=rows_d[ct * P : (ct + 1) * P, n * 512 : (n + 1) * 512],
                        in_=ot[:],
                    )


# ---------------------------------------------------------------------------
# Host-side: build/compile once, shard, run SPMD, unshard.
# ---------------------------------------------------------------------------

_CACHE = {}


def _build():
    import concourse.bacc as bacc
    import concourse.mybir as mybir
    import concourse.tile as tile

    f32 = mybir.dt.float32
    i32 = mybir.dt.int32

    nc = bacc.Bacc("TRN2", target_bir_lowering=False, debug=False,
                   num_devices=N_CORES)

    ins = {
        "x": nc.dram_tensor("x", [T, D], f32, kind="ExternalInput"),
        "wgr": nc.dram_tensor("wgr", [P, DK * E], f32, kind="ExternalInput"),
        "w1": nc.dram_tensor("w1", [D, DFF], mybir.dt.float32r, kind="ExternalInput"),
        "b1c": nc.dram_tensor("b1c", [P, FK], f32, kind="ExternalInput"),
        "w2": nc.dram_tensor("w2", [DFF, D], mybir.dt.float32r, kind="ExternalInput"),
        "b2rep": nc.dram_tensor("b2rep", [P, D], f32, kind="ExternalInput"),
        "selrep": nc.dram_tensor("selrep", [P, E], f32, kind="ExternalInput"),
        "ident": nc.dram_tensor("ident", [P, P], f32, kind="ExternalInput"),
        "u128": nc.dram_tensor("u128", [P, P], f32, kind="ExternalInput"),
        "u32s": nc.dram_tensor("u32s", [32, 32], f32, kind="ExternalInput"),
        "onesp": nc.dram_tensor("onesp", [P, 1], f32, kind="ExternalInput"),
        "selcol": nc.dram_tensor("selcol", [E, 1], f32, kind="ExternalInput"),
    }
    outs = {
        "rows": nc.dram_tensor("rows", [C, D], f32, kind="ExternalOutput"),
        "toks2": nc.dram_tensor("toks2", [C, 2], i32, kind="ExternalOutput"),
        "counts": nc.dram_tensor("counts", [1, E], i32, kind="ExternalOutput"),
        "laux": nc.dram_tensor("laux", [1, 1], f32, kind="ExternalOutput"),
    }
    with tile.TileContext(nc) as tc:
        emit_moe(tc, {k: v.ap() for k, v in outs.items()},
                 {k: v.ap() for k, v in ins.items()})
    nc.compile()
    return nc


def host_inputs(hidden_states, wg, w1, b1, w2, b2):
    """Per-core input maps (host-side shard/layout prep only)."""
    x = np.ascontiguousarray(np.asarray(hidden_states, np.float32).reshape(T, D))
    wg = np.asarray(wg, np.float32)
    w1 = np.asarray(w1, np.float32)
    b1 = np.asarray(b1, np.float32)
    w2 = np.asarray(w2, np.float32)
    b2 = np.asarray(b2, np.float32)

    # wg rearranged k-major: wgr[p, k*E+e] = wg[k*128+p, e]
    wgr = np.ascontiguousarray(
        wg.reshape(DK, P, E).transpose(1, 0, 2).reshape(P, DK * E)
    )
    ident = np.eye(P, dtype=np.float32)
    u128 = np.triu(np.ones((P, P), np.float32))        # incl diag
    u32s = np.triu(np.ones((32, 32), np.float32), 1)   # strictly upper
    onesp = np.ones((P, 1), np.float32)

    in_maps = []
    for e in range(N_CORES):
        b1c = np.ascontiguousarray(b1[e].reshape(FK, P).T)       # [128, FK]
        b2rep = np.ascontiguousarray(np.tile(b2[e][None, :], (P, 1)))
        selrep = np.zeros((P, E), np.float32)
        selrep[:, e] = 1.0
        selcol = np.zeros((E, 1), np.float32)
        selcol[e, 0] = 1.0
        in_maps.append({
            "x": x, "wgr": wgr,
            "w1": np.ascontiguousarray(w1[e]),
            "b1c": b1c,
            "w2": np.ascontiguousarray(w2[e]),
            "b2rep": b2rep, "selrep": selrep,
            "ident": ident, "u128": u128, "u32s": u32s,
            "onesp": onesp, "selcol": selcol,
        })
    return in_maps


def combine(results):
    """Host unshard: scatter each expert's kept rows back to token slots."""
    counts = results[0]["counts"][0].astype(np.int64)
    out = np.zeros((T, D), np.float32)
    for e in range(N_CORES):
        k = int(min(counts[e], C))
        if k > 0:
            tok = results[e]["toks2"][:k, 0].astype(np.int64)
            out[tok] = results[e]["rows"][:k]
    l_aux = np.float32(results[0]["laux"][0, 0])
    exp_counts = results[0]["counts"][0].astype(np.int32)
    return out.reshape(B, S, D), l_aux, exp_counts


def kernel(hidden_states, wg, w1, b1, w2, b2):
    from concourse.bass_utils import run_bass_kernel_spmd

    if "nc" not in _CACHE:
        _CACHE["nc"] = _build()
    nc = _CACHE["nc"]
    in_maps = host_inputs(hidden_states, wg, w1, b1, w2, b2)
    res = run_bass_kernel_spmd(nc, in_maps, list(range(N_CORES)))
    return combine(res.results)


# revision 13
# speedup vs baseline: 1.1026x; 1.1026x over previous
"""Trainium2 Bass kernel for top-1 MoE (nn_MoE_48808008352179).

Expert parallelism across 8 NeuronCores: core e owns expert e's weights.
Full hidden_states is replicated to every core's HBM; each core computes
top-1 routing on-device (logits -> softmax/argmax -> capacity-limited
slot assignment via triangular-matmul cumsum), compacts its own expert's
token list via indirect-DMA scatter, gathers those token rows, runs the
expert MLP (two fp32 GEMMs + tanh-approx gelu) on the tensor engine, and
writes gate-scaled output rows. The host scatters each expert's rows back
to token positions (the "combine"), which is exact because top-1 routing
makes expert outputs disjoint over tokens.

kernel(**inputs) takes the full unsharded inputs and returns
(out[B,S,D] f32, l_aux f32 scalar, exp_counts[E] int32) matching the
reference.
"""

import numpy as np

# Problem dimensions (hardcoded per contest rules).
B, S, D, E, DFF = 2, 2048, 1024, 8, 4096
T = B * S                      # 4096 tokens
C = max(4, -(-T // E))         # 512 capacity (CAP_FACTOR=1.0, MIN_CAP=4)
P = 128                        # partitions
NT = T // P                    # 32 token tiles
DK = D // P                    # 8 contraction tiles over D
FK = DFF // P                  # 32 contraction tiles over DFF
N_CORES = 8

_SLOT_BIG = 65536.0            # sentinel slot for dropped tokens (> C-1 -> skipped)


def emit_moe(tc, outs, ins):
    """Emit the per-core MoE program into TileContext tc.

    ins:  x[T,D], wgr[128,DK*E], w1[D,DFF], b1c[128,FK], w2[DFF,D],
          b2rep[128,D], selrep[128,E], ident[128,128], u128[128,128],
          u32s[32,32], onesp[128,1], selcol[8,1]
    outs: rows[C,D] f32, toks[C,1] i32, gatesl[C,1] f32,
          counts[1,E] i32, laux[1,1] f32
    """
    import concourse.mybir as mybir
    import concourse.bass as bass
    from concourse.tile_rust import add_dep_helper

    nc = tc.nc
    f32 = mybir.dt.float32
    i32 = mybir.dt.int32
    u32 = mybir.dt.uint32
    f32r = mybir.dt.float32r
    AX = mybir.AxisListType
    OP = mybir.AluOpType
    ACTF = mybir.ActivationFunctionType

    x_d = ins["x"]
    wgr_d = ins["wgr"]
    w1_d = ins["w1"]
    b1c_d = ins["b1c"]
    w2_d = ins["w2"]
    b2rep_d = ins["b2rep"]
    selrep_d = ins["selrep"]
    ident_d = ins["ident"]
    u128_d = ins["u128"]
    u32s_d = ins["u32s"]
    onesp_d = ins["onesp"]
    selcol_d = ins["selcol"]

    rows_d = outs["rows"]
    toks2_d = outs["toks2"]
    counts_d = outs["counts"]
    laux_d = outs["laux"]

    from contextlib import ExitStack

    ctx = ExitStack()
    with ctx:
        cst = ctx.enter_context(tc.tile_pool(name="cst", bufs=1))
        per = ctx.enter_context(tc.tile_pool(name="per", bufs=1))
        rt = ctx.enter_context(tc.tile_pool(name="rt", bufs=2))
        dsp = ctx.enter_context(tc.tile_pool(name="dsp", bufs=2))
        w1p = ctx.enter_context(tc.tile_pool(name="w1p", bufs=2))
        w2p = ctx.enter_context(tc.tile_pool(name="w2p", bufs=4))
        outp = ctx.enter_context(tc.tile_pool(name="outp", bufs=3))

        # ---- constants from host ----
        ident = cst.tile([P, P], f32)
        nc.sync.dma_start(out=ident[:], in_=ident_d[:])
        u128 = cst.tile([P, P], f32)
        nc.sync.dma_start(out=u128[:], in_=u128_d[:])
        u32s = cst.tile([32, 32], f32)
        nc.sync.dma_start(out=u32s[:], in_=u32s_d[:])
        wgr = cst.tile([P, DK * E], f32)
        nc.sync.dma_start(out=wgr[:], in_=wgr_d[:])
        selrep = cst.tile([P, E], f32)
        nc.sync.dma_start(out=selrep[:], in_=selrep_d[:])
        sel32 = cst.tile([P, 4 * E], f32)
        for _j in range(4):
            nc.vector.tensor_copy(sel32[:, _j * E : (_j + 1) * E], selrep[:])
        bigslot4 = cst.tile([P, 4], f32)
        nc.vector.memset(bigslot4[:], _SLOT_BIG)
        onesp = cst.tile([P, 1], f32)
        nc.sync.dma_start(out=onesp[:], in_=onesp_d[:])
        selcol = cst.tile([E, 1], f32)
        nc.sync.dma_start(out=selcol[:], in_=selcol_d[:])
        b1c = cst.tile([P, FK], f32)
        nc.sync.dma_start(out=b1c[:], in_=b1c_d[:])
        b2rep = cst.tile([P, D], f32)
        nc.sync.dma_start(out=b2rep[:], in_=b2rep_d[:])

        iota512i = cst.tile([P, C], i32)
        nc.gpsimd.iota(iota512i[:], pattern=[[1, C]], base=0, channel_multiplier=0)
        iota512f = cst.tile([P, C], f32)
        nc.vector.tensor_copy(iota512f[:], iota512i[:])
        iotatoki = cst.tile([P, NT], i32)
        nc.gpsimd.iota(iotatoki[:], pattern=[[P, NT]], base=0, channel_multiplier=1)
        iotatokf = cst.tile([P, NT], f32)
        nc.vector.tensor_copy(iotatokf[:], iotatoki[:])
        iota8i = cst.tile([P, E], i32)
        nc.gpsimd.iota(iota8i[:], pattern=[[1, E]], base=0, channel_multiplier=0)
        iota8 = cst.tile([P, E], f32)
        nc.vector.tensor_copy(iota8[:], iota8i[:])
        big8 = cst.tile([P, E], f32)
        nc.vector.memset(big8[:], float(E))
        bigslot = cst.tile([P, 1], f32)
        nc.vector.memset(bigslot[:], _SLOT_BIG)


        # ---- persistent state ----
        mask_all = per.tile([P, NT * E], f32)
        gate_all = per.tile([P, NT], f32)
        xte = per.tile([P, DK, C], f32r)      # dispatched tokens, transposed
        ht = per.tile([P, FK, C], f32r)       # gelu(x @ w1 + b1), transposed

        # ================= Phase A: logits + per-tile routing =================
        with (
            tc.tile_pool(name="psMe", bufs=1, space="PSUM") as psMe,
            tc.tile_pool(name="psTot", bufs=1, space="PSUM") as psTot,
        ):
            me_ps = psMe.tile([1, E], f32, space="PSUM")
            totT_ps = psTot.tile([E, NT], f32, space="PSUM")

            phaseA = tc.tile_pool(name="psA", bufs=2, space="PSUM")
            psA = phaseA.__enter__()
            phaseL = tc.tile_pool(name="psL", bufs=2, space="PSUM")
            psL = phaseL.__enter__()
            for i in range(NT):
                x_sb = rt.tile([P, D], f32, tag="x_sb")
                nc.sync.dma_start(out=x_sb[:], in_=x_d[i * P : (i + 1) * P, :])

                # transpose x tile -> xT chunks [d, t], batched 4 per psum bank
                xT_sb = rt.tile([P, D], f32, tag="xT_sb")
                for g in range(2):
                    tp = psA.tile([P, 512], f32, space="PSUM", tag="tpA")
                    for j in range(4):
                        k = g * 4 + j
                        nc.tensor.transpose(
                            out=tp[:, j * P : (j + 1) * P],
                            in_=x_sb[:, k * P : (k + 1) * P],
                            identity=ident[:],
                        )
                    if g == 0:
                        nc.scalar.copy(xT_sb[:, g * 512 : (g + 1) * 512], tp[:])
                    else:
                        nc.vector.tensor_copy(
                            xT_sb[:, g * 512 : (g + 1) * 512], tp[:]
                        )

                # logitsT [E, 128] = wg^T @ xT  (wg stationary)
                lgT_ps = psL.tile([E, P], f32, space="PSUM", tag="lgT")
                for k in range(DK):
                    nc.tensor.matmul(
                        lgT_ps[:],
                        lhsT=wgr[:, k * E : (k + 1) * E],
                        rhs=xT_sb[:, k * P : (k + 1) * P],
                        start=(k == 0),
                        stop=(k == DK - 1),
                    )
                lgT_sb = rt.tile([E, P], f32, tag="lgT_sb")
                nc.vector.tensor_copy(lgT_sb[:], lgT_ps[:])
                lg_ps = psL.tile([P, E], f32, space="PSUM", tag="lg")
                nc.tensor.transpose(
                    out=lg_ps[:], in_=lgT_sb[:], identity=ident[:E, :E]
                )
                lg = rt.tile([P, E], f32, tag="lg_sb")
                nc.vector.tensor_copy(lg[:], lg_ps[:])

                # routing math on [128, 8]
                m = rt.tile([P, 1], f32, tag="m")
                nc.vector.tensor_reduce(m[:], lg[:], axis=AX.X, op=OP.max)
                neg_m = rt.tile([P, 1], f32, tag="neg_m")
                nc.scalar.mul(neg_m[:], m[:], -1.0)
                exps = rt.tile([P, E], f32, tag="exps")
                ssum = rt.tile([P, 1], f32, tag="ssum")
                nc.scalar.activation(
                    exps[:], lg[:], ACTF.Exp, bias=neg_m[:], accum_out=ssum[:]
                )
                nc.vector.reciprocal(gate_all[:, i : i + 1], ssum[:])

                eq = rt.tile([P, E], u32, tag="eq")
                nc.vector.tensor_tensor(
                    out=eq[:], in0=lg[:], in1=m[:].to_broadcast([P, E]),
                    op=OP.is_equal,
                )
                masked = rt.tile([P, E], f32, tag="masked")
                nc.vector.select(masked[:], eq[:], iota8[:], big8[:])
                idxf = rt.tile([P, 1], f32, tag="idxf")
                nc.vector.tensor_reduce(idxf[:], masked[:], axis=AX.X, op=OP.min)
                mask_i = mask_all[:, i * E : (i + 1) * E]
                nc.vector.tensor_tensor(
                    out=mask_i, in0=iota8[:], in1=idxf[:].to_broadcast([P, E]),
                    op=OP.is_equal,
                )

                # l_aux numerator: accumulate sum_t softmax probs per expert
                probs = rt.tile([P, E], f32, tag="probs")
                nc.vector.tensor_tensor(
                    out=probs[:], in0=exps[:],
                    in1=gate_all[:, i : i + 1].to_broadcast([P, E]), op=OP.mult,
                )
                nc.tensor.matmul(
                    me_ps[:], lhsT=onesp[:], rhs=probs[:],
                    start=(i == 0), stop=(i == NT - 1), skip_group_check=True,
                )
                # per-tile expert totals into column i of totT [E, NT]
                nc.tensor.matmul(
                    totT_ps[:, i : i + 1], lhsT=mask_i, rhs=onesp[:],
                    start=True, stop=True, skip_group_check=True,
                )

            phaseL.__exit__(None, None, None)
            phaseA.__exit__(None, None, None)

            # ============== Phase B: tile offsets, counts, l_aux ==============
            totT_sb = rt.tile([E, NT], f32)
            nc.vector.tensor_copy(totT_sb[:], totT_ps[:])

            with (
                tc.tile_pool(name="psLoc", bufs=1, space="PSUM") as psLoc,
                tc.tile_pool(name="psTok", bufs=1, space="PSUM") as psTok,
            ):
                tot_ps2 = psLoc.tile([NT, E], f32, space="PSUM", tag="smallB")
                nc.tensor.transpose(
                    out=tot_ps2[:], in_=totT_sb[:], identity=ident[:E, :E]
                )
                tot_sb = rt.tile([NT, E], f32)
                nc.vector.tensor_copy(tot_sb[:], tot_ps2[:])

                offs_ps = psLoc.tile([32, E], f32, space="PSUM", tag="smallB")
                nc.tensor.matmul(
                    offs_ps[:], lhsT=u32s[:], rhs=tot_sb[:], start=True, stop=True
                )
                offs_sb = rt.tile([32, E], f32)
                nc.vector.tensor_copy(offs_sb[:], offs_ps[:])

                # own expert's per-tile offsets, replicated to all partitions
                offsT_ps = psLoc.tile([E, NT], f32, space="PSUM", tag="smallB")
                nc.tensor.transpose(
                    out=offsT_ps[:], in_=offs_sb[:], identity=ident[:NT, :NT]
                )
                offsT_sb = rt.tile([E, NT], f32)
                nc.vector.tensor_copy(offsT_sb[:], offsT_ps[:])
                ownoff_ps = psLoc.tile([1, NT], f32, space="PSUM", tag="smallB")
                nc.tensor.matmul(
                    ownoff_ps[:], lhsT=selcol[:], rhs=offsT_sb[:],
                    start=True, stop=True,
                )
                ownoff_sb = rt.tile([1, NT], f32)
                nc.vector.tensor_copy(ownoff_sb[:], ownoff_ps[:])
                offs_bc = per.tile([P, NT], f32)
                nc.gpsimd.partition_broadcast(offs_bc[:], ownoff_sb[:])

                cnt_ps = psLoc.tile([1, E], f32, space="PSUM", tag="smallB")
                nc.tensor.matmul(
                    cnt_ps[:], lhsT=onesp[:32, :], rhs=tot_sb[:],
                    start=True, stop=True,
                )
                cnt_sb = rt.tile([1, E], f32)
                nc.vector.tensor_copy(cnt_sb[:], cnt_ps[:])
                cnt_i = rt.tile([1, E], i32)
                nc.vector.tensor_copy(cnt_i[:], cnt_sb[:])
                nc.sync.dma_start(out=counts_d[:], in_=cnt_i[:])

                me_sb = rt.tile([1, E], f32)
                nc.vector.tensor_copy(me_sb[:], me_ps[:])
                lx = rt.tile([1, E], f32)
                nc.vector.tensor_tensor(
                    out=lx[:], in0=me_sb[:], in1=cnt_sb[:], op=OP.mult
                )
                lx1 = rt.tile([1, 1], f32)
                nc.vector.tensor_reduce(lx1[:], lx[:], axis=AX.X, op=OP.add)
                laux_sb = rt.tile([1, 1], f32)
                nc.scalar.mul(laux_sb[:], lx1[:], float(E) / float(T) / float(T))
                nc.sync.dma_start(out=laux_d[:], in_=laux_sb[:])

                # ===== Phase C: slots -> (token, gate) per slot via matmuls =====
                # tokmap_ps[c] accumulates [token_id, gate] per slot for chunk
                # c: oh[t, s] = (slot_t == s) selects each kept token's row.
                # One bank per chunk: accumulation groups must not share a bank.
                tokmap_ps = []
                for c in range(C // P):
                    tm = psTok.tile([P, 2], f32, space="PSUM", tag=f"tokmap{c}",
                                    name=f"tokmap_ps_{c}")
                    tokmap_ps.append(tm)
                for i4 in range(NT // 4):
                    msl = mask_all[:, i4 * 4 * E : (i4 + 1) * 4 * E]
                    loc_ps = psLoc.tile([P, 4 * E], f32, space="PSUM", tag="loc")
                    nc.tensor.matmul(
                        loc_ps[:], lhsT=u128[:], rhs=msl, start=True, stop=True
                    )
                    ownm = rt.tile([P, 4 * E], f32, tag="ownm")
                    nc.vector.tensor_tensor(
                        out=ownm[:], in0=msl, in1=sel32[:], op=OP.mult
                    )
                    own1 = rt.tile([P, 4], f32, tag="own1")
                    nc.vector.tensor_reduce(
                        own1[:], ownm[:].rearrange("p (f e) -> p f e", e=E),
                        axis=AX.X, op=OP.add,
                    )
                    lm = rt.tile([P, 4 * E], f32, tag="lm")
                    nc.vector.tensor_tensor(
                        out=lm[:], in0=loc_ps[:], in1=ownm[:], op=OP.mult
                    )
                    own_loc = rt.tile([P, 4], f32, tag="own_loc")
                    nc.vector.tensor_reduce(
                        own_loc[:], lm[:].rearrange("p (f e) -> p f e", e=E),
                        axis=AX.X, op=OP.add,
                    )
                    # global 0-based slot = tile cumsum + tile offset - 1
                    nc.vector.tensor_tensor(
                        out=own_loc[:], in0=own_loc[:],
                        in1=offs_bc[:, i4 * 4 : (i4 + 1) * 4], op=OP.add,
                    )
                    nc.vector.tensor_scalar(
                        out=own_loc[:], in0=own_loc[:], scalar1=1.0, scalar2=None,
                        op0=OP.subtract,
                    )
                    cmp = rt.tile([P, 4], f32, tag="cmp")
                    nc.vector.tensor_scalar(
                        out=cmp[:], in0=own_loc[:], scalar1=float(C), scalar2=None,
                        op0=OP.is_lt,
                    )
                    keep = rt.tile([P, 4], u32, tag="keep")
                    nc.vector.tensor_tensor(
                        out=keep[:], in0=cmp[:], in1=own1[:], op=OP.mult
                    )
                    slotf = rt.tile([P, 4], f32, tag="slotf")
                    nc.vector.select(slotf[:], keep[:], own_loc[:], bigslot4[:])

                    for j in range(4):
                        i = i4 * 4 + j
                        oh = rt.tile([P, C], f32, tag="oh")
                        nc.vector.tensor_tensor(
                            out=oh[:], in0=slotf[:, j : j + 1].to_broadcast([P, C]),
                            in1=iota512f[:], op=OP.is_equal,
                        )
                        rhs2 = rt.tile([P, 2], f32, tag="rhs2")
                        nc.vector.tensor_copy(rhs2[:, 0:1], iotatokf[:, i : i + 1])
                        nc.vector.tensor_copy(rhs2[:, 1:2], gate_all[:, i : i + 1])
                        for c in range(C // P):
                            nc.tensor.matmul(
                                tokmap_ps[c][:],
                                lhsT=oh[:, c * P : (c + 1) * P],
                                rhs=rhs2[:],
                                start=(i == 0), stop=(i == NT - 1),
                                skip_group_check=True,
                            )

                gate_sl = per.tile([P, C // P], f32)
                tok_sl = per.tile([P, C // P], i32)
                packed_out = per.tile([P, C // P, 2], i32)
                for c in range(C // P):
                    nc.vector.tensor_copy(
                        tok_sl[:, c : c + 1], tokmap_ps[c][:, 0:1]
                    )
                    nc.vector.tensor_copy(
                        gate_sl[:, c : c + 1], tokmap_ps[c][:, 1:2]
                    )
                    nc.vector.tensor_copy(
                        packed_out[:, c, 0:1], tok_sl[:, c : c + 1]
                    )
                    nc.vector.tensor_copy(
                        packed_out[:, c, 1:2], gate_sl[:, c : c + 1].bitcast(i32)
                    )

        # ================= Phase D: dispatch gather + transpose ===============
        nc.sync.dma_start(
            out=toks2_d.rearrange("(c p) two -> p c two", p=P), in_=packed_out[:]
        )
        with tc.tile_pool(name="psTr", bufs=4, space="PSUM") as psTr:
            for c in range(C // P):
                xg = dsp.tile([P, D], f32, tag="xg")
                nc.gpsimd.indirect_dma_start(
                    out=xg[:], out_offset=None,
                    in_=x_d[:],
                    in_offset=bass.IndirectOffsetOnAxis(ap=tok_sl[:, c : c + 1], axis=0),
                    bounds_check=T - 1, oob_is_err=False,
                )
                for k in range(DK):
                    tp = psTr.tile([P, P], f32, space="PSUM", tag="tpD")
                    nc.tensor.transpose(
                        out=tp[:], in_=xg[:, k * P : (k + 1) * P], identity=ident[:]
                    )
                    nc.vector.tensor_copy(
                        xte[:, k, c * P : (c + 1) * P], tp[:]
                    )

        # ================= Phase E: GEMM1 + gelu -> hT ========================
        with tc.tile_pool(name="psH", bufs=3, space="PSUM") as psH:
            for g in range(8):          # groups of 4 DFF tiles
                w1g = w1p.tile([P, DK, 512], f32r, tag="w1g")
                for k in range(DK):
                    nc.sync.dma_start(
                        out=w1g[:, k, :],
                        in_=w1_d[k * P : (k + 1) * P, g * 512 : (g + 1) * 512].bitcast(f32r),
                    )
                for mm in range(4):
                    mt = g * 4 + mm
                    h_ps = psH.tile([P, C], f32, space="PSUM", tag="h")
                    for k in range(DK):
                        nc.tensor.matmul(
                            h_ps[:],
                            lhsT=w1g[:, k, mm * P : (mm + 1) * P],
                            rhs=xte[:, k, :],
                            start=(k == 0),
                            stop=(k == DK - 1),
                        )
                    nc.scalar.activation(
                        ht[:, mt, :], h_ps[:], ACTF.Gelu_apprx_tanh,
                        bias=b1c[:, mt : mt + 1],
                    )

        # ================= Phase F: GEMM2 + bias + gate scale =================
        with tc.tile_pool(name="psO", bufs=1, space="PSUM") as psO:
            o_ps = []
            for ct in range(4):
                row = []
                for n in range(2):
                    o_tile = psO.tile([P, 512], f32, space="PSUM", tag=f"o{ct}{n}",
                                      name=f"o_ps_{ct}_{n}")
                    row.append(o_tile)
                o_ps.append(row)
            for k in range(FK):
                w2s = w2p.tile([P, D], f32r, tag="w2s")
                nc.sync.dma_start(
                    out=w2s[:], in_=w2_d[k * P : (k + 1) * P, :].bitcast(f32r))
                for ct in range(4):
                    for n in range(2):
                        nc.tensor.matmul(
                            o_ps[ct][n][:],
                            lhsT=ht[:, k, ct * P : (ct + 1) * P],
                            rhs=w2s[:, n * 512 : (n + 1) * 512],
                            start=(k == 0),
                            stop=(k == FK - 1),
                        )
            for ct in range(4):
                for n in range(2):
                    ot = outp.tile([P, 512], f32, tag="ot")
                    nc.vector.tensor_tensor(
                        out=ot[:], in0=o_ps[ct][n][:],
                        in1=b2rep[:, n * 512 : (n + 1) * 512], op=OP.add,
                    )
                    nc.scalar.mul(ot[:], ot[:], gate_sl[:, ct : ct + 1])
                    nc.sync.dma_start(
                        out=rows_d[ct * P : (ct + 1) * P, n * 512 : (n + 1) * 512],
                        in_=ot[:],
                    )


# ---------------------------------------------------------------------------
# Host-side: build/compile once, shard, run SPMD, unshard.
# ---------------------------------------------------------------------------

_CACHE = {}


def _build():
    import concourse.bacc as bacc
    import concourse.mybir as mybir
    import concourse.tile as tile

    f32 = mybir.dt.float32
    i32 = mybir.dt.int32

    nc = bacc.Bacc("TRN2", target_bir_lowering=False, debug=False,
                   num_devices=N_CORES)

    ins = {
        "x": nc.dram_tensor("x", [T, D], f32, kind="ExternalInput"),
        "wgr": nc.dram_tensor("wgr", [P, DK * E], f32, kind="ExternalInput"),
        "w1": nc.dram_tensor("w1", [D, DFF], mybir.dt.float32r, kind="ExternalInput"),
        "b1c": nc.dram_tensor("b1c", [P, FK], f32, kind="ExternalInput"),
        "w2": nc.dram_tensor("w2", [DFF, D], mybir.dt.float32r, kind="ExternalInput"),
        "b2rep": nc.dram_tensor("b2rep", [P, D], f32, kind="ExternalInput"),
        "selrep": nc.dram_tensor("selrep", [P, E], f32, kind="ExternalInput"),
        "ident": nc.dram_tensor("ident", [P, P], f32, kind="ExternalInput"),
        "u128": nc.dram_tensor("u128", [P, P], f32, kind="ExternalInput"),
        "u32s": nc.dram_tensor("u32s", [32, 32], f32, kind="ExternalInput"),
        "onesp": nc.dram_tensor("onesp", [P, 1], f32, kind="ExternalInput"),
        "selcol": nc.dram_tensor("selcol", [E, 1], f32, kind="ExternalInput"),
    }
    outs = {
        "rows": nc.dram_tensor("rows", [C, D], f32, kind="ExternalOutput"),
        "toks2": nc.dram_tensor("toks2", [C, 2], i32, kind="ExternalOutput"),
        "counts": nc.dram_tensor("counts", [1, E], i32, kind="ExternalOutput"),
        "laux": nc.dram_tensor("laux", [1, 1], f32, kind="ExternalOutput"),
    }
    with tile.TileContext(nc) as tc:
        emit_moe(tc, {k: v.ap() for k, v in outs.items()},
                 {k: v.ap() for k, v in ins.items()})
    nc.compile()
    return nc


def host_inputs(hidden_states, wg, w1, b1, w2, b2):
    """Per-core input maps (host-side shard/layout prep only)."""
    x = np.ascontiguousarray(np.asarray(hidden_states, np.float32).reshape(T, D))
    wg = np.asarray(wg, np.float32)
    w1 = np.asarray(w1, np.float32)
    b1 = np.asarray(b1, np.float32)
    w2 = np.asarray(w2, np.float32)
    b2 = np.asarray(b2, np.float32)

    # wg rearranged k-major: wgr[p, k*E+e] = wg[k*128+p, e]
    wgr = np.ascontiguousarray(
        wg.reshape(DK, P, E).transpose(1, 0, 2).reshape(P, DK * E)
    )
    ident = np.eye(P, dtype=np.float32)
    u128 = np.triu(np.ones((P, P), np.float32))        # incl diag
    u32s = np.triu(np.ones((32, 32), np.float32), 1)   # strictly upper
    onesp = np.ones((P, 1), np.float32)

    in_maps = []
    for e in range(N_CORES):
        b1c = np.ascontiguousarray(b1[e].reshape(FK, P).T)       # [128, FK]
        b2rep = np.ascontiguousarray(np.tile(b2[e][None, :], (P, 1)))
        selrep = np.zeros((P, E), np.float32)
        selrep[:, e] = 1.0
        selcol = np.zeros((E, 1), np.float32)
        selcol[e, 0] = 1.0
        in_maps.append({
            "x": x, "wgr": wgr,
            "w1": np.ascontiguousarray(w1[e]),
            "b1c": b1c,
            "w2": np.ascontiguousarray(w2[e]),
            "b2rep": b2rep, "selrep": selrep,
            "ident": ident, "u128": u128, "u32s": u32s,
            "onesp": onesp, "selcol": selcol,
        })
    return in_maps


def combine(results):
    """Host unshard: scatter each expert's kept rows back to token slots."""
    counts = results[0]["counts"][0].astype(np.int64)
    out = np.zeros((T, D), np.float32)
    for e in range(N_CORES):
        k = int(min(counts[e], C))
        if k > 0:
            tok = results[e]["toks2"][:k, 0].astype(np.int64)
            out[tok] = results[e]["rows"][:k]
    l_aux = np.float32(results[0]["laux"][0, 0])
    exp_counts = results[0]["counts"][0].astype(np.int32)
    return out.reshape(B, S, D), l_aux, exp_counts


def kernel(hidden_states, wg, w1, b1, w2, b2):
    from concourse.bass_utils import run_bass_kernel_spmd

    if "nc" not in _CACHE:
        _CACHE["nc"] = _build()
    nc = _CACHE["nc"]
    in_maps = host_inputs(hidden_states, wg, w1, b1, w2, b2)
    res = run_bass_kernel_spmd(nc, in_maps, list(range(N_CORES)))
    return combine(res.results)


# revision 14
# speedup vs baseline: 1.1572x; 1.0496x over previous
"""Trainium2 Bass kernel for top-1 MoE (nn_MoE_48808008352179).

Expert parallelism across 8 NeuronCores: core e owns expert e's weights.
Full hidden_states is replicated to every core's HBM; each core computes
top-1 routing on-device (logits -> softmax/argmax -> capacity-limited
slot assignment via triangular-matmul cumsum), compacts its own expert's
token list via indirect-DMA scatter, gathers those token rows, runs the
expert MLP (two fp32 GEMMs + tanh-approx gelu) on the tensor engine, and
writes gate-scaled output rows. The host scatters each expert's rows back
to token positions (the "combine"), which is exact because top-1 routing
makes expert outputs disjoint over tokens.

kernel(**inputs) takes the full unsharded inputs and returns
(out[B,S,D] f32, l_aux f32 scalar, exp_counts[E] int32) matching the
reference.
"""

import numpy as np

# Problem dimensions (hardcoded per contest rules).
B, S, D, E, DFF = 2, 2048, 1024, 8, 4096
T = B * S                      # 4096 tokens
C = max(4, -(-T // E))         # 512 capacity (CAP_FACTOR=1.0, MIN_CAP=4)
P = 128                        # partitions
NT = T // P                    # 32 token tiles
DK = D // P                    # 8 contraction tiles over D
FK = DFF // P                  # 32 contraction tiles over DFF
N_CORES = 8

_SLOT_BIG = 65536.0            # sentinel slot for dropped tokens (> C-1 -> skipped)


def emit_moe(tc, outs, ins):
    """Emit the per-core MoE program into TileContext tc.

    ins:  x[T,D], wgr[128,DK*E], w1[D,DFF], b1c[128,FK], w2[DFF,D],
          b2rep[128,D], selrep[128,E], ident[128,128], u128[128,128],
          u32s[32,32], onesp[128,1], selcol[8,1]
    outs: rows[C,D] f32, toks[C,1] i32, gatesl[C,1] f32,
          counts[1,E] i32, laux[1,1] f32
    """
    import concourse.mybir as mybir
    import concourse.bass as bass
    from concourse.tile_rust import add_dep_helper

    nc = tc.nc
    f32 = mybir.dt.float32
    i32 = mybir.dt.int32
    u32 = mybir.dt.uint32
    f32r = mybir.dt.float32r
    AX = mybir.AxisListType
    OP = mybir.AluOpType
    ACTF = mybir.ActivationFunctionType

    x_d = ins["x"]
    wgr_d = ins["wgr"]
    w1_d = ins["w1"]
    b1c_d = ins["b1c"]
    w2_d = ins["w2"]
    b2rep_d = ins["b2rep"]
    selrep_d = ins["selrep"]
    ident_d = ins["ident"]
    u128_d = ins["u128"]
    u32s_d = ins["u32s"]
    onesp_d = ins["onesp"]
    selcol_d = ins["selcol"]

    rows_d = outs["rows"]
    toks2_d = outs["toks2"]
    counts_d = outs["counts"]
    laux_d = outs["laux"]

    from contextlib import ExitStack

    ctx = ExitStack()
    with ctx:
        cst = ctx.enter_context(tc.tile_pool(name="cst", bufs=1))
        per = ctx.enter_context(tc.tile_pool(name="per", bufs=1))
        rt = ctx.enter_context(tc.tile_pool(name="rt", bufs=3))
        dsp = ctx.enter_context(tc.tile_pool(name="dsp", bufs=2))
        w1p = ctx.enter_context(tc.tile_pool(name="w1p", bufs=2))
        w2p = ctx.enter_context(tc.tile_pool(name="w2p", bufs=6))
        outp = ctx.enter_context(tc.tile_pool(name="outp", bufs=3))

        # ---- constants from host ----
        ident = cst.tile([P, P], f32)
        nc.sync.dma_start(out=ident[:], in_=ident_d[:])
        u128 = cst.tile([P, P], f32)
        nc.sync.dma_start(out=u128[:], in_=u128_d[:])
        u32s = cst.tile([32, 32], f32)
        nc.sync.dma_start(out=u32s[:], in_=u32s_d[:])
        wgr = cst.tile([P, DK * E], f32)
        nc.sync.dma_start(out=wgr[:], in_=wgr_d[:])
        selrep = cst.tile([P, E], f32)
        nc.sync.dma_start(out=selrep[:], in_=selrep_d[:])
        sel32 = cst.tile([P, 4 * E], f32)
        for _j in range(4):
            nc.vector.tensor_copy(sel32[:, _j * E : (_j + 1) * E], selrep[:])
        bigslot4 = cst.tile([P, 4], f32)
        nc.vector.memset(bigslot4[:], _SLOT_BIG)
        onesp = cst.tile([P, 1], f32)
        nc.sync.dma_start(out=onesp[:], in_=onesp_d[:])
        selcol = cst.tile([E, 1], f32)
        nc.sync.dma_start(out=selcol[:], in_=selcol_d[:])
        b1c = cst.tile([P, FK], f32)
        nc.sync.dma_start(out=b1c[:], in_=b1c_d[:])
        b2rep = cst.tile([P, D], f32)
        nc.sync.dma_start(out=b2rep[:], in_=b2rep_d[:])

        iota512i = cst.tile([P, C], i32)
        nc.gpsimd.iota(iota512i[:], pattern=[[1, C]], base=0, channel_multiplier=0)
        iota512f = cst.tile([P, C], f32)
        nc.vector.tensor_copy(iota512f[:], iota512i[:])
        iotatoki = cst.tile([P, NT], i32)
        nc.gpsimd.iota(iotatoki[:], pattern=[[P, NT]], base=0, channel_multiplier=1)
        iotatokf = cst.tile([P, NT], f32)
        nc.vector.tensor_copy(iotatokf[:], iotatoki[:])
        iota8i = cst.tile([P, E], i32)
        nc.gpsimd.iota(iota8i[:], pattern=[[1, E]], base=0, channel_multiplier=0)
        iota8 = cst.tile([P, E], f32)
        nc.vector.tensor_copy(iota8[:], iota8i[:])
        big8 = cst.tile([P, E], f32)
        nc.vector.memset(big8[:], float(E))
        bigslot = cst.tile([P, 1], f32)
        nc.vector.memset(bigslot[:], _SLOT_BIG)


        # ---- persistent state ----
        mask_all = per.tile([P, NT * E], f32)
        gate_all = per.tile([P, NT], f32)
        xte = per.tile([P, DK, C], f32r)      # dispatched tokens, transposed
        ht = per.tile([P, FK, C], f32r)       # gelu(x @ w1 + b1), transposed

        # ================= Phase A: logits + per-tile routing =================
        with (
            tc.tile_pool(name="psMe", bufs=1, space="PSUM") as psMe,
            tc.tile_pool(name="psTot", bufs=1, space="PSUM") as psTot,
        ):
            me_ps = psMe.tile([1, E], f32, space="PSUM")
            totT_ps = psTot.tile([E, NT], f32, space="PSUM")

            phaseA = tc.tile_pool(name="psA", bufs=2, space="PSUM")
            psA = phaseA.__enter__()
            phaseL = tc.tile_pool(name="psL", bufs=2, space="PSUM")
            psL = phaseL.__enter__()
            for i in range(NT):
                x_sb = rt.tile([P, D], f32, tag="x_sb")
                nc.sync.dma_start(out=x_sb[:], in_=x_d[i * P : (i + 1) * P, :])

                # transpose x tile -> xT chunks [d, t], batched 4 per psum bank
                xT_sb = rt.tile([P, D], f32, tag="xT_sb")
                for g in range(2):
                    tp = psA.tile([P, 512], f32, space="PSUM", tag="tpA")
                    for j in range(4):
                        k = g * 4 + j
                        nc.tensor.transpose(
                            out=tp[:, j * P : (j + 1) * P],
                            in_=x_sb[:, k * P : (k + 1) * P],
                            identity=ident[:],
                        )
                    if g == 0:
                        nc.scalar.copy(xT_sb[:, g * 512 : (g + 1) * 512], tp[:])
                    else:
                        nc.vector.tensor_copy(
                            xT_sb[:, g * 512 : (g + 1) * 512], tp[:]
                        )

                # logitsT [E, 128] = wg^T @ xT  (wg stationary)
                lgT_ps = psL.tile([E, P], f32, space="PSUM", tag="lgT")
                for k in range(DK):
                    nc.tensor.matmul(
                        lgT_ps[:],
                        lhsT=wgr[:, k * E : (k + 1) * E],
                        rhs=xT_sb[:, k * P : (k + 1) * P],
                        start=(k == 0),
                        stop=(k == DK - 1),
                    )
                lgT_sb = rt.tile([E, P], f32, tag="lgT_sb")
                nc.vector.tensor_copy(lgT_sb[:], lgT_ps[:])
                lg_ps = psL.tile([P, E], f32, space="PSUM", tag="lg")
                nc.tensor.transpose(
                    out=lg_ps[:], in_=lgT_sb[:], identity=ident[:E, :E]
                )
                lg = rt.tile([P, E], f32, tag="lg_sb")
                nc.vector.tensor_copy(lg[:], lg_ps[:])

                # routing math on [128, 8]
                m = rt.tile([P, 1], f32, tag="m")
                nc.vector.tensor_reduce(m[:], lg[:], axis=AX.X, op=OP.max)
                neg_m = rt.tile([P, 1], f32, tag="neg_m")
                nc.scalar.mul(neg_m[:], m[:], -1.0)
                exps = rt.tile([P, E], f32, tag="exps")
                ssum = rt.tile([P, 1], f32, tag="ssum")
                nc.scalar.activation(
                    exps[:], lg[:], ACTF.Exp, bias=neg_m[:], accum_out=ssum[:]
                )
                nc.vector.reciprocal(gate_all[:, i : i + 1], ssum[:])

                eq = rt.tile([P, E], u32, tag="eq")
                nc.vector.tensor_tensor(
                    out=eq[:], in0=lg[:], in1=m[:].to_broadcast([P, E]),
                    op=OP.is_equal,
                )
                masked = rt.tile([P, E], f32, tag="masked")
                nc.vector.select(masked[:], eq[:], iota8[:], big8[:])
                idxf = rt.tile([P, 1], f32, tag="idxf")
                nc.vector.tensor_reduce(idxf[:], masked[:], axis=AX.X, op=OP.min)
                mask_i = mask_all[:, i * E : (i + 1) * E]
                nc.vector.tensor_tensor(
                    out=mask_i, in0=iota8[:], in1=idxf[:].to_broadcast([P, E]),
                    op=OP.is_equal,
                )

                # l_aux numerator: accumulate sum_t softmax probs per expert
                probs = rt.tile([P, E], f32, tag="probs")
                nc.vector.tensor_tensor(
                    out=probs[:], in0=exps[:],
                    in1=gate_all[:, i : i + 1].to_broadcast([P, E]), op=OP.mult,
                )
                nc.tensor.matmul(
                    me_ps[:], lhsT=onesp[:], rhs=probs[:],
                    start=(i == 0), stop=(i == NT - 1), skip_group_check=True,
                )
                # per-tile expert totals into column i of totT [E, NT]
                nc.tensor.matmul(
                    totT_ps[:, i : i + 1], lhsT=mask_i, rhs=onesp[:],
                    start=True, stop=True, skip_group_check=True,
                )

            phaseL.__exit__(None, None, None)
            phaseA.__exit__(None, None, None)

            # ============== Phase B: tile offsets, counts, l_aux ==============
            totT_sb = rt.tile([E, NT], f32)
            nc.vector.tensor_copy(totT_sb[:], totT_ps[:])

            with (
                tc.tile_pool(name="psLoc", bufs=1, space="PSUM") as psLoc,
                tc.tile_pool(name="psTok", bufs=1, space="PSUM") as psTok,
            ):
                tot_ps2 = psLoc.tile([NT, E], f32, space="PSUM", tag="smallB")
                nc.tensor.transpose(
                    out=tot_ps2[:], in_=totT_sb[:], identity=ident[:E, :E]
                )
                tot_sb = rt.tile([NT, E], f32)
                nc.vector.tensor_copy(tot_sb[:], tot_ps2[:])

                offs_ps = psLoc.tile([32, E], f32, space="PSUM", tag="smallB")
                nc.tensor.matmul(
                    offs_ps[:], lhsT=u32s[:], rhs=tot_sb[:], start=True, stop=True
                )
                offs_sb = rt.tile([32, E], f32)
                nc.vector.tensor_copy(offs_sb[:], offs_ps[:])

                # own expert's per-tile offsets, replicated to all partitions
                offsT_ps = psLoc.tile([E, NT], f32, space="PSUM", tag="smallB")
                nc.tensor.transpose(
                    out=offsT_ps[:], in_=offs_sb[:], identity=ident[:NT, :NT]
                )
                offsT_sb = rt.tile([E, NT], f32)
                nc.vector.tensor_copy(offsT_sb[:], offsT_ps[:])
                ownoff_ps = psLoc.tile([1, NT], f32, space="PSUM", tag="smallB")
                nc.tensor.matmul(
                    ownoff_ps[:], lhsT=selcol[:], rhs=offsT_sb[:],
                    start=True, stop=True,
                )
                ownoff_sb = rt.tile([1, NT], f32)
                nc.vector.tensor_copy(ownoff_sb[:], ownoff_ps[:])
                offs_bc = per.tile([P, NT], f32)
                nc.gpsimd.partition_broadcast(offs_bc[:], ownoff_sb[:])

                cnt_ps = psLoc.tile([1, E], f32, space="PSUM", tag="smallB")
                nc.tensor.matmul(
                    cnt_ps[:], lhsT=onesp[:32, :], rhs=tot_sb[:],
                    start=True, stop=True,
                )
                cnt_sb = rt.tile([1, E], f32)
                nc.vector.tensor_copy(cnt_sb[:], cnt_ps[:])
                cnt_i = rt.tile([1, E], i32)
                nc.vector.tensor_copy(cnt_i[:], cnt_sb[:])
                nc.sync.dma_start(out=counts_d[:], in_=cnt_i[:])

                me_sb = rt.tile([1, E], f32)
                nc.vector.tensor_copy(me_sb[:], me_ps[:])
                lx = rt.tile([1, E], f32)
                nc.vector.tensor_tensor(
                    out=lx[:], in0=me_sb[:], in1=cnt_sb[:], op=OP.mult
                )
                lx1 = rt.tile([1, 1], f32)
                nc.vector.tensor_reduce(lx1[:], lx[:], axis=AX.X, op=OP.add)
                laux_sb = rt.tile([1, 1], f32)
                nc.scalar.mul(laux_sb[:], lx1[:], float(E) / float(T) / float(T))
                nc.sync.dma_start(out=laux_d[:], in_=laux_sb[:])

                # ===== Phase C: slots -> (token, gate) per slot via matmuls =====
                # tokmap_ps[c] accumulates [token_id, gate] per slot for chunk
                # c: oh[t, s] = (slot_t == s) selects each kept token's row.
                # One bank per chunk: accumulation groups must not share a bank.
                tokmap_ps = []
                for c in range(C // P):
                    tm = psTok.tile([P, 2], f32, space="PSUM", tag=f"tokmap{c}",
                                    name=f"tokmap_ps_{c}")
                    tokmap_ps.append(tm)
                for i4 in range(NT // 4):
                    msl = mask_all[:, i4 * 4 * E : (i4 + 1) * 4 * E]
                    loc_ps = psLoc.tile([P, 4 * E], f32, space="PSUM", tag="loc")
                    nc.tensor.matmul(
                        loc_ps[:], lhsT=u128[:], rhs=msl, start=True, stop=True
                    )
                    ownm = rt.tile([P, 4 * E], f32, tag="ownm")
                    nc.vector.tensor_tensor(
                        out=ownm[:], in0=msl, in1=sel32[:], op=OP.mult
                    )
                    own1 = rt.tile([P, 4], f32, tag="own1")
                    nc.vector.tensor_reduce(
                        own1[:], ownm[:].rearrange("p (f e) -> p f e", e=E),
                        axis=AX.X, op=OP.add,
                    )
                    lm = rt.tile([P, 4 * E], f32, tag="lm")
                    nc.vector.tensor_tensor(
                        out=lm[:], in0=loc_ps[:], in1=ownm[:], op=OP.mult
                    )
                    own_loc = rt.tile([P, 4], f32, tag="own_loc")
                    nc.vector.tensor_reduce(
                        own_loc[:], lm[:].rearrange("p (f e) -> p f e", e=E),
                        axis=AX.X, op=OP.add,
                    )
                    # global 0-based slot = tile cumsum + tile offset - 1
                    nc.vector.tensor_tensor(
                        out=own_loc[:], in0=own_loc[:],
                        in1=offs_bc[:, i4 * 4 : (i4 + 1) * 4], op=OP.add,
                    )
                    nc.vector.tensor_scalar(
                        out=own_loc[:], in0=own_loc[:], scalar1=1.0, scalar2=None,
                        op0=OP.subtract,
                    )
                    cmp = rt.tile([P, 4], f32, tag="cmp")
                    nc.vector.tensor_scalar(
                        out=cmp[:], in0=own_loc[:], scalar1=float(C), scalar2=None,
                        op0=OP.is_lt,
                    )
                    keep = rt.tile([P, 4], u32, tag="keep")
                    nc.vector.tensor_tensor(
                        out=keep[:], in0=cmp[:], in1=own1[:], op=OP.mult
                    )
                    slotf = rt.tile([P, 4], f32, tag="slotf")
                    nc.vector.select(slotf[:], keep[:], own_loc[:], bigslot4[:])

                    for j in range(4):
                        i = i4 * 4 + j
                        oh = rt.tile([P, C], f32, tag="oh")
                        nc.vector.tensor_tensor(
                            out=oh[:], in0=slotf[:, j : j + 1].to_broadcast([P, C]),
                            in1=iota512f[:], op=OP.is_equal,
                        )
                        rhs2 = rt.tile([P, 2], f32, tag="rhs2")
                        nc.vector.tensor_copy(rhs2[:, 0:1], iotatokf[:, i : i + 1])
                        nc.vector.tensor_copy(rhs2[:, 1:2], gate_all[:, i : i + 1])
                        for c in range(C // P):
                            nc.tensor.matmul(
                                tokmap_ps[c][:],
                                lhsT=oh[:, c * P : (c + 1) * P],
                                rhs=rhs2[:],
                                start=(i == 0), stop=(i == NT - 1),
                                skip_group_check=True,
                            )

                gate_sl = per.tile([P, C // P], f32)
                tok_sl = per.tile([P, C // P], i32)
                packed_out = per.tile([P, C // P, 2], i32)
                for c in range(C // P):
                    nc.vector.tensor_copy(
                        tok_sl[:, c : c + 1], tokmap_ps[c][:, 0:1]
                    )
                    nc.vector.tensor_copy(
                        gate_sl[:, c : c + 1], tokmap_ps[c][:, 1:2]
                    )
                    nc.vector.tensor_copy(
                        packed_out[:, c, 0:1], tok_sl[:, c : c + 1]
                    )
                    nc.vector.tensor_copy(
                        packed_out[:, c, 1:2], gate_sl[:, c : c + 1].bitcast(i32)
                    )

        # ================= Phase D: dispatch gather + transpose ===============
        nc.sync.dma_start(
            out=toks2_d.rearrange("(c p) two -> p c two", p=P), in_=packed_out[:]
        )
        with tc.tile_pool(name="psTr", bufs=4, space="PSUM") as psTr:
            for c in range(C // P):
                xg = dsp.tile([P, D], f32, tag="xg")
                nc.gpsimd.indirect_dma_start(
                    out=xg[:], out_offset=None,
                    in_=x_d[:],
                    in_offset=bass.IndirectOffsetOnAxis(ap=tok_sl[:, c : c + 1], axis=0),
                    bounds_check=T - 1, oob_is_err=False,
                )
                for k in range(DK):
                    tp = psTr.tile([P, P], f32, space="PSUM", tag="tpD")
                    nc.tensor.transpose(
                        out=tp[:], in_=xg[:, k * P : (k + 1) * P], identity=ident[:]
                    )
                    nc.vector.tensor_copy(
                        xte[:, k, c * P : (c + 1) * P], tp[:]
                    )

        # ================= Phase E: GEMM1 + gelu -> hT ========================
        with tc.tile_pool(name="psH", bufs=3, space="PSUM") as psH:
            for g in range(8):          # groups of 4 DFF tiles
                w1g = w1p.tile([P, DK, 512], f32r, tag="w1g")
                for k in range(DK):
                    nc.sync.dma_start(
                        out=w1g[:, k, :],
                        in_=w1_d[k * P : (k + 1) * P, g * 512 : (g + 1) * 512].bitcast(f32r),
                    )
                for mm in range(4):
                    mt = g * 4 + mm
                    h_ps = psH.tile([P, C], f32, space="PSUM", tag="h")
                    for k in range(DK):
                        nc.tensor.matmul(
                            h_ps[:],
                            lhsT=w1g[:, k, mm * P : (mm + 1) * P],
                            rhs=xte[:, k, :],
                            start=(k == 0),
                            stop=(k == DK - 1),
                        )
                    nc.scalar.activation(
                        ht[:, mt, :], h_ps[:], ACTF.Gelu_apprx_tanh,
                        bias=b1c[:, mt : mt + 1],
                    )

        # ================= Phase F: GEMM2 + bias + gate scale =================
        with tc.tile_pool(name="psO", bufs=1, space="PSUM") as psO:
            o_ps = []
            for ct in range(4):
                row = []
                for n in range(2):
                    o_tile = psO.tile([P, 512], f32, space="PSUM", tag=f"o{ct}{n}",
                                      name=f"o_ps_{ct}_{n}")
                    row.append(o_tile)
                o_ps.append(row)
            for k in range(FK):
                w2s = w2p.tile([P, D], f32r, tag="w2s")
                nc.sync.dma_start(
                    out=w2s[:], in_=w2_d[k * P : (k + 1) * P, :].bitcast(f32r))
                for ct in range(4):
                    for n in range(2):
                        nc.tensor.matmul(
                            o_ps[ct][n][:],
                            lhsT=ht[:, k, ct * P : (ct + 1) * P],
                            rhs=w2s[:, n * 512 : (n + 1) * 512],
                            start=(k == 0),
                            stop=(k == FK - 1),
                        )
            for ct in range(4):
                for n in range(2):
                    ot = outp.tile([P, 512], f32, tag="ot")
                    nc.vector.tensor_tensor(
                        out=ot[:], in0=o_ps[ct][n][:],
                        in1=b2rep[:, n * 512 : (n + 1) * 512], op=OP.add,
                    )
                    nc.scalar.mul(ot[:], ot[:], gate_sl[:, ct : ct + 1])
                    nc.sync.dma_start(
                        out=rows_d[ct * P : (ct + 1) * P, n * 512 : (n + 1) * 512],
                        in_=ot[:],
                    )


# ---------------------------------------------------------------------------
# Host-side: build/compile once, shard, run SPMD, unshard.
# ---------------------------------------------------------------------------

_CACHE = {}


def _build():
    import concourse.bacc as bacc
    import concourse.mybir as mybir
    import concourse.tile as tile

    f32 = mybir.dt.float32
    i32 = mybir.dt.int32

    nc = bacc.Bacc("TRN2", target_bir_lowering=False, debug=False,
                   num_devices=N_CORES)

    ins = {
        "x": nc.dram_tensor("x", [T, D], f32, kind="ExternalInput"),
        "wgr": nc.dram_tensor("wgr", [P, DK * E], f32, kind="ExternalInput"),
        "w1": nc.dram_tensor("w1", [D, DFF], mybir.dt.float32r, kind="ExternalInput"),
        "b1c": nc.dram_tensor("b1c", [P, FK], f32, kind="ExternalInput"),
        "w2": nc.dram_tensor("w2", [DFF, D], mybir.dt.float32r, kind="ExternalInput"),
        "b2rep": nc.dram_tensor("b2rep", [P, D], f32, kind="ExternalInput"),
        "selrep": nc.dram_tensor("selrep", [P, E], f32, kind="ExternalInput"),
        "ident": nc.dram_tensor("ident", [P, P], f32, kind="ExternalInput"),
        "u128": nc.dram_tensor("u128", [P, P], f32, kind="ExternalInput"),
        "u32s": nc.dram_tensor("u32s", [32, 32], f32, kind="ExternalInput"),
        "onesp": nc.dram_tensor("onesp", [P, 1], f32, kind="ExternalInput"),
        "selcol": nc.dram_tensor("selcol", [E, 1], f32, kind="ExternalInput"),
    }
    outs = {
        "rows": nc.dram_tensor("rows", [C, D], f32, kind="ExternalOutput"),
        "toks2": nc.dram_tensor("toks2", [C, 2], i32, kind="ExternalOutput"),
        "counts": nc.dram_tensor("counts", [1, E], i32, kind="ExternalOutput"),
        "laux": nc.dram_tensor("laux", [1, 1], f32, kind="ExternalOutput"),
    }
    with tile.TileContext(nc) as tc:
        emit_moe(tc, {k: v.ap() for k, v in outs.items()},
                 {k: v.ap() for k, v in ins.items()})
    nc.compile()
    return nc


def host_inputs(hidden_states, wg, w1, b1, w2, b2):
    """Per-core input maps (host-side shard/layout prep only)."""
    x = np.ascontiguousarray(np.asarray(hidden_states, np.float32).reshape(T, D))
    wg = np.asarray(wg, np.float32)
    w1 = np.asarray(w1, np.float32)
    b1 = np.asarray(b1, np.float32)
    w2 = np.asarray(w2, np.float32)
    b2 = np.asarray(b2, np.float32)

    # wg rearranged k-major: wgr[p, k*E+e] = wg[k*128+p, e]
    wgr = np.ascontiguousarray(
        wg.reshape(DK, P, E).transpose(1, 0, 2).reshape(P, DK * E)
    )
    ident = np.eye(P, dtype=np.float32)
    u128 = np.triu(np.ones((P, P), np.float32))        # incl diag
    u32s = np.triu(np.ones((32, 32), np.float32), 1)   # strictly upper
    onesp = np.ones((P, 1), np.float32)

    in_maps = []
    for e in range(N_CORES):
        b1c = np.ascontiguousarray(b1[e].reshape(FK, P).T)       # [128, FK]
        b2rep = np.ascontiguousarray(np.tile(b2[e][None, :], (P, 1)))
        selrep = np.zeros((P, E), np.float32)
        selrep[:, e] = 1.0
        selcol = np.zeros((E, 1), np.float32)
        selcol[e, 0] = 1.0
        in_maps.append({
            "x": x, "wgr": wgr,
            "w1": np.ascontiguousarray(w1[e]),
            "b1c": b1c,
            "w2": np.ascontiguousarray(w2[e]),
            "b2rep": b2rep, "selrep": selrep,
            "ident": ident, "u128": u128, "u32s": u32s,
            "onesp": onesp, "selcol": selcol,
        })
    return in_maps


def combine(results):
    """Host unshard: scatter each expert's kept rows back to token slots."""
    counts = results[0]["counts"][0].astype(np.int64)
    out = np.zeros((T, D), np.float32)
    for e in range(N_CORES):
        k = int(min(counts[e], C))
        if k > 0:
            tok = results[e]["toks2"][:k, 0].astype(np.int64)
            out[tok] = results[e]["rows"][:k]
    l_aux = np.float32(results[0]["laux"][0, 0])
    exp_counts = results[0]["counts"][0].astype(np.int32)
    return out.reshape(B, S, D), l_aux, exp_counts


def kernel(hidden_states, wg, w1, b1, w2, b2):
    from concourse.bass_utils import run_bass_kernel_spmd

    if "nc" not in _CACHE:
        _CACHE["nc"] = _build()
    nc = _CACHE["nc"]
    in_maps = host_inputs(hidden_states, wg, w1, b1, w2, b2)
    res = run_bass_kernel_spmd(nc, in_maps, list(range(N_CORES)))
    return combine(res.results)


# revision 15
# speedup vs baseline: 1.2391x; 1.0707x over previous
"""Trainium2 Bass kernel for top-1 MoE (nn_MoE_48808008352179).

Expert parallelism across 8 NeuronCores: core e owns expert e's weights.
Full hidden_states is replicated to every core's HBM; each core computes
top-1 routing on-device (logits -> softmax/argmax -> capacity-limited
slot assignment via triangular-matmul cumsum), compacts its own expert's
token list via indirect-DMA scatter, gathers those token rows, runs the
expert MLP (two fp32 GEMMs + tanh-approx gelu) on the tensor engine, and
writes gate-scaled output rows. The host scatters each expert's rows back
to token positions (the "combine"), which is exact because top-1 routing
makes expert outputs disjoint over tokens.

kernel(**inputs) takes the full unsharded inputs and returns
(out[B,S,D] f32, l_aux f32 scalar, exp_counts[E] int32) matching the
reference.
"""

import numpy as np

# Problem dimensions (hardcoded per contest rules).
B, S, D, E, DFF = 2, 2048, 1024, 8, 4096
T = B * S                      # 4096 tokens
C = max(4, -(-T // E))         # 512 capacity (CAP_FACTOR=1.0, MIN_CAP=4)
P = 128                        # partitions
NT = T // P                    # 32 token tiles
DK = D // P                    # 8 contraction tiles over D
FK = DFF // P                  # 32 contraction tiles over DFF
N_CORES = 8

_SLOT_BIG = 65536.0            # sentinel slot for dropped tokens (> C-1 -> skipped)


def emit_moe(tc, outs, ins):
    """Emit the per-core MoE program into TileContext tc.

    ins:  x[T,D], wgr[128,DK*E], w1[D,DFF], b1c[128,FK], w2[DFF,D],
          b2rep[128,D], selrep[128,E], ident[128,128], u128[128,128],
          u32s[32,32], onesp[128,1], selcol[8,1]
    outs: rows[C,D] f32, toks[C,1] i32, gatesl[C,1] f32,
          counts[1,E] i32, laux[1,1] f32
    """
    import concourse.mybir as mybir
    import concourse.bass as bass
    from concourse.tile_rust import add_dep_helper

    nc = tc.nc
    f32 = mybir.dt.float32
    i32 = mybir.dt.int32
    u32 = mybir.dt.uint32
    f32r = mybir.dt.float32r
    AX = mybir.AxisListType
    OP = mybir.AluOpType
    ACTF = mybir.ActivationFunctionType

    x_d = ins["x"]
    wgr_d = ins["wgr"]
    w1_d = ins["w1"]
    b1c_d = ins["b1c"]
    w2_d = ins["w2"]
    b2rep_d = ins["b2rep"]
    selrep_d = ins["selrep"]
    ident_d = ins["ident"]
    u128_d = ins["u128"]
    u32s_d = ins["u32s"]
    onesp_d = ins["onesp"]
    selcol_d = ins["selcol"]

    rows_d = outs["rows"]
    toks2_d = outs["toks2"]
    counts_d = outs["counts"]
    laux_d = outs["laux"]

    from contextlib import ExitStack

    ctx = ExitStack()
    with ctx:
        cst = ctx.enter_context(tc.tile_pool(name="cst", bufs=1))
        per = ctx.enter_context(tc.tile_pool(name="per", bufs=1))
        rt = ctx.enter_context(tc.tile_pool(name="rt", bufs=3))
        dsp = ctx.enter_context(tc.tile_pool(name="dsp", bufs=2))
        w1p = ctx.enter_context(tc.tile_pool(name="w1p", bufs=2))
        w2p = ctx.enter_context(tc.tile_pool(name="w2p", bufs=6))
        outp = ctx.enter_context(tc.tile_pool(name="outp", bufs=3))

        # ---- constants from host ----
        ident = cst.tile([P, P], f32)
        nc.sync.dma_start(out=ident[:], in_=ident_d[:])
        u128 = cst.tile([P, P], f32)
        nc.sync.dma_start(out=u128[:], in_=u128_d[:])
        u32s = cst.tile([32, 32], f32)
        nc.sync.dma_start(out=u32s[:], in_=u32s_d[:])
        wgr = cst.tile([P, DK * E], f32)
        nc.sync.dma_start(out=wgr[:], in_=wgr_d[:])
        selrep = cst.tile([P, E], f32)
        nc.sync.dma_start(out=selrep[:], in_=selrep_d[:])
        sel32 = cst.tile([P, 4 * E], f32)
        for _j in range(4):
            nc.vector.tensor_copy(sel32[:, _j * E : (_j + 1) * E], selrep[:])
        bigslot4 = cst.tile([P, 4], f32)
        nc.vector.memset(bigslot4[:], _SLOT_BIG)
        onesp = cst.tile([P, 1], f32)
        nc.sync.dma_start(out=onesp[:], in_=onesp_d[:])
        selcol = cst.tile([E, 1], f32)
        nc.sync.dma_start(out=selcol[:], in_=selcol_d[:])
        b1c = cst.tile([P, FK], f32)
        nc.sync.dma_start(out=b1c[:], in_=b1c_d[:])
        b2rep = cst.tile([P, D], f32)
        nc.sync.dma_start(out=b2rep[:], in_=b2rep_d[:])

        iota512i = cst.tile([P, C], i32)
        nc.gpsimd.iota(iota512i[:], pattern=[[1, C]], base=0, channel_multiplier=0)
        iota512f = cst.tile([P, C], f32)
        nc.vector.tensor_copy(iota512f[:], iota512i[:])
        iotatoki = cst.tile([P, NT], i32)
        nc.gpsimd.iota(iotatoki[:], pattern=[[P, NT]], base=0, channel_multiplier=1)
        iotatokf = cst.tile([P, NT], f32)
        nc.vector.tensor_copy(iotatokf[:], iotatoki[:])
        iota8i = cst.tile([P, E], i32)
        nc.gpsimd.iota(iota8i[:], pattern=[[1, E]], base=0, channel_multiplier=0)
        iota8 = cst.tile([P, E], f32)
        nc.vector.tensor_copy(iota8[:], iota8i[:])
        big8 = cst.tile([P, E], f32)
        nc.vector.memset(big8[:], float(E))
        bigslot = cst.tile([P, 1], f32)
        nc.vector.memset(bigslot[:], _SLOT_BIG)


        # ---- persistent state ----
        mask_all = per.tile([P, NT * E], f32)
        gate_all = per.tile([P, NT], f32)
        xte = per.tile([P, DK, C], f32r)      # dispatched tokens, transposed
        ht = per.tile([P, FK, C], f32r)       # gelu(x @ w1 + b1), transposed

        # ================= Phase A: logits + per-tile routing =================
        with (
            tc.tile_pool(name="psMe", bufs=1, space="PSUM") as psMe,
            tc.tile_pool(name="psTot", bufs=1, space="PSUM") as psTot,
        ):
            me_ps = psMe.tile([1, E], f32, space="PSUM")
            totT_ps = psTot.tile([E, NT], f32, space="PSUM")

            phaseA = tc.tile_pool(name="psA", bufs=2, space="PSUM")
            psA = phaseA.__enter__()
            phaseL = tc.tile_pool(name="psL", bufs=2, space="PSUM")
            psL = phaseL.__enter__()
            for i in range(NT):
                x_sb = rt.tile([P, D], f32, tag="x_sb")
                nc.sync.dma_start(out=x_sb[:], in_=x_d[i * P : (i + 1) * P, :])

                # transpose x tile -> xT chunks [d, t], batched 4 per psum bank
                xT_sb = rt.tile([P, D], f32, tag="xT_sb")
                for g in range(2):
                    tp = psA.tile([P, 512], f32, space="PSUM", tag="tpA")
                    for j in range(4):
                        k = g * 4 + j
                        nc.tensor.transpose(
                            out=tp[:, j * P : (j + 1) * P],
                            in_=x_sb[:, k * P : (k + 1) * P],
                            identity=ident[:],
                        )
                    if g == 0:
                        nc.scalar.copy(xT_sb[:, g * 512 : (g + 1) * 512], tp[:])
                    else:
                        nc.vector.tensor_copy(
                            xT_sb[:, g * 512 : (g + 1) * 512], tp[:]
                        )

                # logitsT [E, 128] = wg^T @ xT  (wg stationary)
                lgT_ps = psL.tile([E, P], f32, space="PSUM", tag="lgT")
                for k in range(DK):
                    nc.tensor.matmul(
                        lgT_ps[:],
                        lhsT=wgr[:, k * E : (k + 1) * E],
                        rhs=xT_sb[:, k * P : (k + 1) * P],
                        start=(k == 0),
                        stop=(k == DK - 1),
                    )
                lgT_sb = rt.tile([E, P], f32, tag="lgT_sb")
                nc.vector.tensor_copy(lgT_sb[:], lgT_ps[:])
                lg_ps = psL.tile([P, E], f32, space="PSUM", tag="lg")
                nc.tensor.transpose(
                    out=lg_ps[:], in_=lgT_sb[:], identity=ident[:E, :E]
                )
                lg = rt.tile([P, E], f32, tag="lg_sb")
                nc.vector.tensor_copy(lg[:], lg_ps[:])

                # routing math on [128, 8]
                m = rt.tile([P, 1], f32, tag="m")
                nc.vector.tensor_reduce(m[:], lg[:], axis=AX.X, op=OP.max)
                neg_m = rt.tile([P, 1], f32, tag="neg_m")
                nc.scalar.mul(neg_m[:], m[:], -1.0)
                exps = rt.tile([P, E], f32, tag="exps")
                ssum = rt.tile([P, 1], f32, tag="ssum")
                nc.scalar.activation(
                    exps[:], lg[:], ACTF.Exp, bias=neg_m[:], accum_out=ssum[:]
                )
                nc.vector.reciprocal(gate_all[:, i : i + 1], ssum[:])

                eq = rt.tile([P, E], u32, tag="eq")
                nc.vector.tensor_tensor(
                    out=eq[:], in0=lg[:], in1=m[:].to_broadcast([P, E]),
                    op=OP.is_equal,
                )
                masked = rt.tile([P, E], f32, tag="masked")
                nc.vector.select(masked[:], eq[:], iota8[:], big8[:])
                idxf = rt.tile([P, 1], f32, tag="idxf")
                nc.vector.tensor_reduce(idxf[:], masked[:], axis=AX.X, op=OP.min)
                mask_i = mask_all[:, i * E : (i + 1) * E]
                nc.vector.tensor_tensor(
                    out=mask_i, in0=iota8[:], in1=idxf[:].to_broadcast([P, E]),
                    op=OP.is_equal,
                )

                # l_aux numerator: accumulate sum_t softmax probs per expert
                probs = rt.tile([P, E], f32, tag="probs")
                nc.vector.tensor_tensor(
                    out=probs[:], in0=exps[:],
                    in1=gate_all[:, i : i + 1].to_broadcast([P, E]), op=OP.mult,
                )
                nc.tensor.matmul(
                    me_ps[:], lhsT=onesp[:], rhs=probs[:],
                    start=(i == 0), stop=(i == NT - 1), skip_group_check=True,
                )
                # per-tile expert totals into column i of totT [E, NT]
                nc.tensor.matmul(
                    totT_ps[:, i : i + 1], lhsT=mask_i, rhs=onesp[:],
                    start=True, stop=True, skip_group_check=True,
                )

            phaseL.__exit__(None, None, None)
            phaseA.__exit__(None, None, None)

            # ============== Phase B: tile offsets, counts, l_aux ==============
            totT_sb = rt.tile([E, NT], f32)
            nc.vector.tensor_copy(totT_sb[:], totT_ps[:])

            with (
                tc.tile_pool(name="psLoc", bufs=1, space="PSUM") as psLoc,
                tc.tile_pool(name="psTok", bufs=2, space="PSUM") as psTok,
            ):
                tot_ps2 = psLoc.tile([NT, E], f32, space="PSUM", tag="smallB")
                nc.tensor.transpose(
                    out=tot_ps2[:], in_=totT_sb[:], identity=ident[:E, :E]
                )
                tot_sb = rt.tile([NT, E], f32)
                nc.vector.tensor_copy(tot_sb[:], tot_ps2[:])

                offs_ps = psLoc.tile([32, E], f32, space="PSUM", tag="smallB")
                nc.tensor.matmul(
                    offs_ps[:], lhsT=u32s[:], rhs=tot_sb[:], start=True, stop=True
                )
                offs_sb = rt.tile([32, E], f32)
                nc.vector.tensor_copy(offs_sb[:], offs_ps[:])

                # own expert's per-tile offsets, replicated to all partitions
                offsT_ps = psLoc.tile([E, NT], f32, space="PSUM", tag="smallB")
                nc.tensor.transpose(
                    out=offsT_ps[:], in_=offs_sb[:], identity=ident[:NT, :NT]
                )
                offsT_sb = rt.tile([E, NT], f32)
                nc.vector.tensor_copy(offsT_sb[:], offsT_ps[:])
                ownoff_ps = psLoc.tile([1, NT], f32, space="PSUM", tag="smallB")
                nc.tensor.matmul(
                    ownoff_ps[:], lhsT=selcol[:], rhs=offsT_sb[:],
                    start=True, stop=True,
                )
                ownoff_sb = rt.tile([1, NT], f32)
                nc.vector.tensor_copy(ownoff_sb[:], ownoff_ps[:])
                offs_bc = per.tile([P, NT], f32)
                nc.gpsimd.partition_broadcast(offs_bc[:], ownoff_sb[:])

                cnt_ps = psLoc.tile([1, E], f32, space="PSUM", tag="smallB")
                nc.tensor.matmul(
                    cnt_ps[:], lhsT=onesp[:32, :], rhs=tot_sb[:],
                    start=True, stop=True,
                )
                cnt_sb = rt.tile([1, E], f32)
                nc.vector.tensor_copy(cnt_sb[:], cnt_ps[:])
                cnt_i = rt.tile([1, E], i32)
                nc.vector.tensor_copy(cnt_i[:], cnt_sb[:])
                nc.sync.dma_start(out=counts_d[:], in_=cnt_i[:])

                me_sb = rt.tile([1, E], f32)
                nc.vector.tensor_copy(me_sb[:], me_ps[:])
                lx = rt.tile([1, E], f32)
                nc.vector.tensor_tensor(
                    out=lx[:], in0=me_sb[:], in1=cnt_sb[:], op=OP.mult
                )
                lx1 = rt.tile([1, 1], f32)
                nc.vector.tensor_reduce(lx1[:], lx[:], axis=AX.X, op=OP.add)
                laux_sb = rt.tile([1, 1], f32)
                nc.scalar.mul(laux_sb[:], lx1[:], float(E) / float(T) / float(T))
                nc.sync.dma_start(out=laux_d[:], in_=laux_sb[:])

                # ===== Phase C: slots -> (token, gate) per slot via matmuls =====
                # tokmapT accumulates [token_id, gate] x slots transposed:
                # lhsT = [token,gate] pair (2-col LDW, ~free), rhs = slot
                # indicator oh (N=512 moving) -> one matmul per token tile.
                tokmapT_ps = psTok.tile([2, C], f32, space="PSUM", tag="tokmapT")
                for i4 in range(NT // 4):
                    msl = mask_all[:, i4 * 4 * E : (i4 + 1) * 4 * E]
                    loc_ps = psLoc.tile([P, 4 * E], f32, space="PSUM", tag="loc")
                    nc.tensor.matmul(
                        loc_ps[:], lhsT=u128[:], rhs=msl, start=True, stop=True
                    )
                    ownm = rt.tile([P, 4 * E], f32, tag="ownm")
                    nc.vector.tensor_tensor(
                        out=ownm[:], in0=msl, in1=sel32[:], op=OP.mult
                    )
                    own1 = rt.tile([P, 4], f32, tag="own1")
                    nc.vector.tensor_reduce(
                        own1[:], ownm[:].rearrange("p (f e) -> p f e", e=E),
                        axis=AX.X, op=OP.add,
                    )
                    lm = rt.tile([P, 4 * E], f32, tag="lm")
                    nc.vector.tensor_tensor(
                        out=lm[:], in0=loc_ps[:], in1=ownm[:], op=OP.mult
                    )
                    own_loc = rt.tile([P, 4], f32, tag="own_loc")
                    nc.vector.tensor_reduce(
                        own_loc[:], lm[:].rearrange("p (f e) -> p f e", e=E),
                        axis=AX.X, op=OP.add,
                    )
                    # global 0-based slot = tile cumsum + tile offset - 1
                    nc.vector.tensor_tensor(
                        out=own_loc[:], in0=own_loc[:],
                        in1=offs_bc[:, i4 * 4 : (i4 + 1) * 4], op=OP.add,
                    )
                    nc.vector.tensor_scalar(
                        out=own_loc[:], in0=own_loc[:], scalar1=1.0, scalar2=None,
                        op0=OP.subtract,
                    )
                    cmp = rt.tile([P, 4], f32, tag="cmp")
                    nc.vector.tensor_scalar(
                        out=cmp[:], in0=own_loc[:], scalar1=float(C), scalar2=None,
                        op0=OP.is_lt,
                    )
                    keep = rt.tile([P, 4], u32, tag="keep")
                    nc.vector.tensor_tensor(
                        out=keep[:], in0=cmp[:], in1=own1[:], op=OP.mult
                    )
                    slotf = rt.tile([P, 4], f32, tag="slotf")
                    nc.vector.select(slotf[:], keep[:], own_loc[:], bigslot4[:])

                    for j in range(4):
                        i = i4 * 4 + j
                        oh = rt.tile([P, C], f32, tag="oh")
                        nc.vector.tensor_tensor(
                            out=oh[:], in0=slotf[:, j : j + 1].to_broadcast([P, C]),
                            in1=iota512f[:], op=OP.is_equal,
                        )
                        rhs2 = rt.tile([P, 2], f32, tag="rhs2")
                        nc.vector.tensor_copy(rhs2[:, 0:1], iotatokf[:, i : i + 1])
                        nc.vector.tensor_copy(rhs2[:, 1:2], gate_all[:, i : i + 1])
                        nc.tensor.matmul(
                            tokmapT_ps[:], lhsT=rhs2[:], rhs=oh[:],
                            start=(i == 0), stop=(i == NT - 1),
                        )

                tokmapT_sb = rt.tile([2, C], f32)
                nc.vector.tensor_copy(tokmapT_sb[:], tokmapT_ps[:])
                gate_sl = per.tile([P, C // P], f32)
                tok_sl = per.tile([P, C // P], i32)
                packed_out = per.tile([P, C // P, 2], i32)
                for c in range(C // P):
                    tm_ps = psTok.tile([P, 2], f32, space="PSUM", tag="tmT")
                    nc.tensor.transpose(
                        out=tm_ps[:], in_=tokmapT_sb[:, c * P : (c + 1) * P],
                        identity=ident[:2, :2],
                    )
                    nc.vector.tensor_copy(
                        tok_sl[:, c : c + 1], tm_ps[:, 0:1]
                    )
                    nc.vector.tensor_copy(
                        gate_sl[:, c : c + 1], tm_ps[:, 1:2]
                    )
                    nc.vector.tensor_copy(
                        packed_out[:, c, 0:1], tok_sl[:, c : c + 1]
                    )
                    nc.vector.tensor_copy(
                        packed_out[:, c, 1:2], gate_sl[:, c : c + 1].bitcast(i32)
                    )

        # ================= Phase D: dispatch gather + transpose ===============
        nc.sync.dma_start(
            out=toks2_d.rearrange("(c p) two -> p c two", p=P), in_=packed_out[:]
        )
        with tc.tile_pool(name="psTr", bufs=4, space="PSUM") as psTr:
            for c in range(C // P):
                xg = dsp.tile([P, D], f32, tag="xg")
                nc.gpsimd.indirect_dma_start(
                    out=xg[:], out_offset=None,
                    in_=x_d[:],
                    in_offset=bass.IndirectOffsetOnAxis(ap=tok_sl[:, c : c + 1], axis=0),
                    bounds_check=T - 1, oob_is_err=False,
                )
                for k in range(DK):
                    tp = psTr.tile([P, P], f32, space="PSUM", tag="tpD")
                    nc.tensor.transpose(
                        out=tp[:], in_=xg[:, k * P : (k + 1) * P], identity=ident[:]
                    )
                    nc.vector.tensor_copy(
                        xte[:, k, c * P : (c + 1) * P], tp[:]
                    )

        # ================= Phase E: GEMM1 + gelu -> hT ========================
        with tc.tile_pool(name="psH", bufs=3, space="PSUM") as psH:
            for g in range(8):          # groups of 4 DFF tiles
                w1g = w1p.tile([P, DK, 512], f32r, tag="w1g")
                for k in range(DK):
                    nc.sync.dma_start(
                        out=w1g[:, k, :],
                        in_=w1_d[k * P : (k + 1) * P, g * 512 : (g + 1) * 512].bitcast(f32r),
                    )
                for mm in range(4):
                    mt = g * 4 + mm
                    h_ps = psH.tile([P, C], f32, space="PSUM", tag="h")
                    for k in range(DK):
                        nc.tensor.matmul(
                            h_ps[:],
                            lhsT=w1g[:, k, mm * P : (mm + 1) * P],
                            rhs=xte[:, k, :],
                            start=(k == 0),
                            stop=(k == DK - 1),
                        )
                    nc.scalar.activation(
                        ht[:, mt, :], h_ps[:], ACTF.Gelu_apprx_tanh,
                        bias=b1c[:, mt : mt + 1],
                    )

        # ================= Phase F: GEMM2 + bias + gate scale =================
        with tc.tile_pool(name="psO", bufs=1, space="PSUM") as psO:
            o_ps = []
            for ct in range(4):
                row = []
                for n in range(2):
                    o_tile = psO.tile([P, 512], f32, space="PSUM", tag=f"o{ct}{n}",
                                      name=f"o_ps_{ct}_{n}")
                    row.append(o_tile)
                o_ps.append(row)
            for k in range(FK):
                w2s = w2p.tile([P, D], f32r, tag="w2s")
                nc.sync.dma_start(
                    out=w2s[:], in_=w2_d[k * P : (k + 1) * P, :].bitcast(f32r))
                for ct in range(4):
                    for n in range(2):
                        nc.tensor.matmul(
                            o_ps[ct][n][:],
                            lhsT=ht[:, k, ct * P : (ct + 1) * P],
                            rhs=w2s[:, n * 512 : (n + 1) * 512],
                            start=(k == 0),
                            stop=(k == FK - 1),
                        )
            for ct in range(4):
                for n in range(2):
                    ot = outp.tile([P, 512], f32, tag="ot")
                    nc.vector.tensor_tensor(
                        out=ot[:], in0=o_ps[ct][n][:],
                        in1=b2rep[:, n * 512 : (n + 1) * 512], op=OP.add,
                    )
                    nc.scalar.mul(ot[:], ot[:], gate_sl[:, ct : ct + 1])
                    nc.sync.dma_start(
                        out=rows_d[ct * P : (ct + 1) * P, n * 512 : (n + 1) * 512],
                        in_=ot[:],
                    )


# ---------------------------------------------------------------------------
# Host-side: build/compile once, shard, run SPMD, unshard.
# ---------------------------------------------------------------------------

_CACHE = {}


def _build():
    import concourse.bacc as bacc
    import concourse.mybir as mybir
    import concourse.tile as tile

    f32 = mybir.dt.float32
    i32 = mybir.dt.int32

    nc = bacc.Bacc("TRN2", target_bir_lowering=False, debug=False,
                   num_devices=N_CORES)

    ins = {
        "x": nc.dram_tensor("x", [T, D], f32, kind="ExternalInput"),
        "wgr": nc.dram_tensor("wgr", [P, DK * E], f32, kind="ExternalInput"),
        "w1": nc.dram_tensor("w1", [D, DFF], mybir.dt.float32r, kind="ExternalInput"),
        "b1c": nc.dram_tensor("b1c", [P, FK], f32, kind="ExternalInput"),
        "w2": nc.dram_tensor("w2", [DFF, D], mybir.dt.float32r, kind="ExternalInput"),
        "b2rep": nc.dram_tensor("b2rep", [P, D], f32, kind="ExternalInput"),
        "selrep": nc.dram_tensor("selrep", [P, E], f32, kind="ExternalInput"),
        "ident": nc.dram_tensor("ident", [P, P], f32, kind="ExternalInput"),
        "u128": nc.dram_tensor("u128", [P, P], f32, kind="ExternalInput"),
        "u32s": nc.dram_tensor("u32s", [32, 32], f32, kind="ExternalInput"),
        "onesp": nc.dram_tensor("onesp", [P, 1], f32, kind="ExternalInput"),
        "selcol": nc.dram_tensor("selcol", [E, 1], f32, kind="ExternalInput"),
    }
    outs = {
        "rows": nc.dram_tensor("rows", [C, D], f32, kind="ExternalOutput"),
        "toks2": nc.dram_tensor("toks2", [C, 2], i32, kind="ExternalOutput"),
        "counts": nc.dram_tensor("counts", [1, E], i32, kind="ExternalOutput"),
        "laux": nc.dram_tensor("laux", [1, 1], f32, kind="ExternalOutput"),
    }
    with tile.TileContext(nc) as tc:
        emit_moe(tc, {k: v.ap() for k, v in outs.items()},
                 {k: v.ap() for k, v in ins.items()})
    nc.compile()
    return nc


def host_inputs(hidden_states, wg, w1, b1, w2, b2):
    """Per-core input maps (host-side shard/layout prep only)."""
    x = np.ascontiguousarray(np.asarray(hidden_states, np.float32).reshape(T, D))
    wg = np.asarray(wg, np.float32)
    w1 = np.asarray(w1, np.float32)
    b1 = np.asarray(b1, np.float32)
    w2 = np.asarray(w2, np.float32)
    b2 = np.asarray(b2, np.float32)

    # wg rearranged k-major: wgr[p, k*E+e] = wg[k*128+p, e]
    wgr = np.ascontiguousarray(
        wg.reshape(DK, P, E).transpose(1, 0, 2).reshape(P, DK * E)
    )
    ident = np.eye(P, dtype=np.float32)
    u128 = np.triu(np.ones((P, P), np.float32))        # incl diag
    u32s = np.triu(np.ones((32, 32), np.float32), 1)   # strictly upper
    onesp = np.ones((P, 1), np.float32)

    in_maps = []
    for e in range(N_CORES):
        b1c = np.ascontiguousarray(b1[e].reshape(FK, P).T)       # [128, FK]
        b2rep = np.ascontiguousarray(np.tile(b2[e][None, :], (P, 1)))
        selrep = np.zeros((P, E), np.float32)
        selrep[:, e] = 1.0
        selcol = np.zeros((E, 1), np.float32)
        selcol[e, 0] = 1.0
        in_maps.append({
            "x": x, "wgr": wgr,
            "w1": np.ascontiguousarray(w1[e]),
            "b1c": b1c,
            "w2": np.ascontiguousarray(w2[e]),
            "b2rep": b2rep, "selrep": selrep,
            "ident": ident, "u128": u128, "u32s": u32s,
            "onesp": onesp, "selcol": selcol,
        })
    return in_maps


def combine(results):
    """Host unshard: scatter each expert's kept rows back to token slots."""
    counts = results[0]["counts"][0].astype(np.int64)
    out = np.zeros((T, D), np.float32)
    for e in range(N_CORES):
        k = int(min(counts[e], C))
        if k > 0:
            tok = results[e]["toks2"][:k, 0].astype(np.int64)
            out[tok] = results[e]["rows"][:k]
    l_aux = np.float32(results[0]["laux"][0, 0])
    exp_counts = results[0]["counts"][0].astype(np.int32)
    return out.reshape(B, S, D), l_aux, exp_counts


def kernel(hidden_states, wg, w1, b1, w2, b2):
    from concourse.bass_utils import run_bass_kernel_spmd

    if "nc" not in _CACHE:
        _CACHE["nc"] = _build()
    nc = _CACHE["nc"]
    in_maps = host_inputs(hidden_states, wg, w1, b1, w2, b2)
    res = run_bass_kernel_spmd(nc, in_maps, list(range(N_CORES)))
    return combine(res.results)


# revision 16
# speedup vs baseline: 1.2583x; 1.0156x over previous
"""Trainium2 Bass kernel for top-1 MoE (nn_MoE_48808008352179).

Expert parallelism across 8 NeuronCores: core e owns expert e's weights.
Full hidden_states is replicated to every core's HBM; each core computes
top-1 routing on-device (logits -> softmax/argmax -> capacity-limited
slot assignment via triangular-matmul cumsum), compacts its own expert's
token list via indirect-DMA scatter, gathers those token rows, runs the
expert MLP (two fp32 GEMMs + tanh-approx gelu) on the tensor engine, and
writes gate-scaled output rows. The host scatters each expert's rows back
to token positions (the "combine"), which is exact because top-1 routing
makes expert outputs disjoint over tokens.

kernel(**inputs) takes the full unsharded inputs and returns
(out[B,S,D] f32, l_aux f32 scalar, exp_counts[E] int32) matching the
reference.
"""

import numpy as np

# Problem dimensions (hardcoded per contest rules).
B, S, D, E, DFF = 2, 2048, 1024, 8, 4096
T = B * S                      # 4096 tokens
C = max(4, -(-T // E))         # 512 capacity (CAP_FACTOR=1.0, MIN_CAP=4)
P = 128                        # partitions
NT = T // P                    # 32 token tiles
DK = D // P                    # 8 contraction tiles over D
FK = DFF // P                  # 32 contraction tiles over DFF
N_CORES = 8

_SLOT_BIG = 65536.0            # sentinel slot for dropped tokens (> C-1 -> skipped)


def emit_moe(tc, outs, ins):
    """Emit the per-core MoE program into TileContext tc.

    ins:  x[T,D], wgr[128,DK*E], w1[D,DFF], b1c[128,FK], w2[DFF,D],
          b2rep[128,D], selrep[128,E], ident[128,128], u128[128,128],
          u32s[32,32], onesp[128,1], selcol[8,1]
    outs: rows[C,D] f32, toks[C,1] i32, gatesl[C,1] f32,
          counts[1,E] i32, laux[1,1] f32
    """
    import concourse.mybir as mybir
    import concourse.bass as bass
    from concourse.tile_rust import add_dep_helper

    nc = tc.nc
    f32 = mybir.dt.float32
    i32 = mybir.dt.int32
    u32 = mybir.dt.uint32
    f32r = mybir.dt.float32r
    AX = mybir.AxisListType
    OP = mybir.AluOpType
    ACTF = mybir.ActivationFunctionType

    x_d = ins["x"]
    wgr_d = ins["wgr"]
    w1_d = ins["w1"]
    b1c_d = ins["b1c"]
    w2_d = ins["w2"]
    b2rep_d = ins["b2rep"]
    selrep_d = ins["selrep"]
    ident_d = ins["ident"]
    u128_d = ins["u128"]
    u32s_d = ins["u32s"]
    onesp_d = ins["onesp"]
    selcol_d = ins["selcol"]

    rows_d = outs["rows"]
    toks2_d = outs["toks2"]
    counts_d = outs["counts"]
    laux_d = outs["laux"]

    from contextlib import ExitStack

    ctx = ExitStack()
    with ctx:
        cst = ctx.enter_context(tc.tile_pool(name="cst", bufs=1))
        per = ctx.enter_context(tc.tile_pool(name="per", bufs=1))
        rt = ctx.enter_context(tc.tile_pool(name="rt", bufs=3))
        dsp = ctx.enter_context(tc.tile_pool(name="dsp", bufs=3))
        w1p = ctx.enter_context(tc.tile_pool(name="w1p", bufs=2))
        w2p = ctx.enter_context(tc.tile_pool(name="w2p", bufs=6))
        outp = ctx.enter_context(tc.tile_pool(name="outp", bufs=3))

        # ---- constants from host ----
        ident = cst.tile([P, P], f32)
        nc.sync.dma_start(out=ident[:], in_=ident_d[:])
        u128 = cst.tile([P, P], f32)
        nc.sync.dma_start(out=u128[:], in_=u128_d[:])
        u32s = cst.tile([32, 32], f32)
        nc.sync.dma_start(out=u32s[:], in_=u32s_d[:])
        wgr = cst.tile([P, DK * E], f32)
        nc.sync.dma_start(out=wgr[:], in_=wgr_d[:])
        selrep = cst.tile([P, E], f32)
        nc.sync.dma_start(out=selrep[:], in_=selrep_d[:])
        sel32 = cst.tile([P, 4 * E], f32)
        for _j in range(4):
            nc.vector.tensor_copy(sel32[:, _j * E : (_j + 1) * E], selrep[:])
        bigslot4 = cst.tile([P, 4], f32)
        nc.vector.memset(bigslot4[:], _SLOT_BIG)
        onesp = cst.tile([P, 1], f32)
        nc.sync.dma_start(out=onesp[:], in_=onesp_d[:])
        selcol = cst.tile([E, 1], f32)
        nc.sync.dma_start(out=selcol[:], in_=selcol_d[:])
        b1c = cst.tile([P, FK], f32)
        nc.sync.dma_start(out=b1c[:], in_=b1c_d[:])
        b2rep = cst.tile([P, D], f32)
        nc.sync.dma_start(out=b2rep[:], in_=b2rep_d[:])

        iota512i = cst.tile([P, C], i32)
        nc.gpsimd.iota(iota512i[:], pattern=[[1, C]], base=0, channel_multiplier=0)
        iota512f = cst.tile([P, C], f32)
        nc.vector.tensor_copy(iota512f[:], iota512i[:])
        iotatoki = cst.tile([P, NT], i32)
        nc.gpsimd.iota(iotatoki[:], pattern=[[P, NT]], base=0, channel_multiplier=1)
        iotatokf = cst.tile([P, NT], f32)
        nc.vector.tensor_copy(iotatokf[:], iotatoki[:])
        iota8i = cst.tile([P, E], i32)
        nc.gpsimd.iota(iota8i[:], pattern=[[1, E]], base=0, channel_multiplier=0)
        iota8 = cst.tile([P, E], f32)
        nc.vector.tensor_copy(iota8[:], iota8i[:])
        big8 = cst.tile([P, E], f32)
        nc.vector.memset(big8[:], float(E))
        bigslot = cst.tile([P, 1], f32)
        nc.vector.memset(bigslot[:], _SLOT_BIG)


        # ---- persistent state ----
        mask_all = per.tile([P, NT * E], f32)
        gate_all = per.tile([P, NT], f32)
        xte = per.tile([P, DK, C], f32r)      # dispatched tokens, transposed
        ht = per.tile([P, FK, C], f32r)       # gelu(x @ w1 + b1), transposed

        # ================= Phase A: logits + per-tile routing =================
        with (
            tc.tile_pool(name="psMe", bufs=1, space="PSUM") as psMe,
            tc.tile_pool(name="psTot", bufs=1, space="PSUM") as psTot,
        ):
            me_ps = psMe.tile([1, E], f32, space="PSUM")
            totT_ps = psTot.tile([E, NT], f32, space="PSUM")

            phaseA = tc.tile_pool(name="psA", bufs=2, space="PSUM")
            psA = phaseA.__enter__()
            phaseL = tc.tile_pool(name="psL", bufs=2, space="PSUM")
            psL = phaseL.__enter__()
            for i in range(NT):
                x_sb = rt.tile([P, D], f32, tag="x_sb")
                nc.sync.dma_start(out=x_sb[:], in_=x_d[i * P : (i + 1) * P, :])

                # transpose x tile -> xT chunks [d, t], batched 4 per psum bank
                xT_sb = rt.tile([P, D], f32, tag="xT_sb")
                for g in range(2):
                    tp = psA.tile([P, 512], f32, space="PSUM", tag="tpA")
                    for j in range(4):
                        k = g * 4 + j
                        nc.tensor.transpose(
                            out=tp[:, j * P : (j + 1) * P],
                            in_=x_sb[:, k * P : (k + 1) * P],
                            identity=ident[:],
                        )
                    if g == 0:
                        nc.scalar.copy(xT_sb[:, g * 512 : (g + 1) * 512], tp[:])
                    else:
                        nc.vector.tensor_copy(
                            xT_sb[:, g * 512 : (g + 1) * 512], tp[:]
                        )

                # logitsT [E, 128] = wg^T @ xT  (wg stationary)
                lgT_ps = psL.tile([E, P], f32, space="PSUM", tag="lgT")
                for k in range(DK):
                    nc.tensor.matmul(
                        lgT_ps[:],
                        lhsT=wgr[:, k * E : (k + 1) * E],
                        rhs=xT_sb[:, k * P : (k + 1) * P],
                        start=(k == 0),
                        stop=(k == DK - 1),
                    )
                lgT_sb = rt.tile([E, P], f32, tag="lgT_sb")
                nc.vector.tensor_copy(lgT_sb[:], lgT_ps[:])
                lg_ps = psL.tile([P, E], f32, space="PSUM", tag="lg")
                nc.tensor.transpose(
                    out=lg_ps[:], in_=lgT_sb[:], identity=ident[:E, :E]
                )
                lg = rt.tile([P, E], f32, tag="lg_sb")
                nc.vector.tensor_copy(lg[:], lg_ps[:])

                # routing math on [128, 8]
                m = rt.tile([P, 1], f32, tag="m")
                nc.vector.tensor_reduce(m[:], lg[:], axis=AX.X, op=OP.max)
                neg_m = rt.tile([P, 1], f32, tag="neg_m")
                nc.scalar.mul(neg_m[:], m[:], -1.0)
                exps = rt.tile([P, E], f32, tag="exps")
                ssum = rt.tile([P, 1], f32, tag="ssum")
                nc.scalar.activation(
                    exps[:], lg[:], ACTF.Exp, bias=neg_m[:], accum_out=ssum[:]
                )
                nc.vector.reciprocal(gate_all[:, i : i + 1], ssum[:])

                eq = rt.tile([P, E], u32, tag="eq")
                nc.vector.tensor_tensor(
                    out=eq[:], in0=lg[:], in1=m[:].to_broadcast([P, E]),
                    op=OP.is_equal,
                )
                masked = rt.tile([P, E], f32, tag="masked")
                nc.vector.select(masked[:], eq[:], iota8[:], big8[:])
                idxf = rt.tile([P, 1], f32, tag="idxf")
                nc.vector.tensor_reduce(idxf[:], masked[:], axis=AX.X, op=OP.min)
                mask_i = mask_all[:, i * E : (i + 1) * E]
                nc.vector.tensor_tensor(
                    out=mask_i, in0=iota8[:], in1=idxf[:].to_broadcast([P, E]),
                    op=OP.is_equal,
                )

                # l_aux numerator: accumulate sum_t softmax probs per expert
                probs = rt.tile([P, E], f32, tag="probs")
                nc.vector.tensor_tensor(
                    out=probs[:], in0=exps[:],
                    in1=gate_all[:, i : i + 1].to_broadcast([P, E]), op=OP.mult,
                )
                nc.tensor.matmul(
                    me_ps[:], lhsT=onesp[:], rhs=probs[:],
                    start=(i == 0), stop=(i == NT - 1), skip_group_check=True,
                )
                # per-tile expert totals into column i of totT [E, NT]
                nc.tensor.matmul(
                    totT_ps[:, i : i + 1], lhsT=mask_i, rhs=onesp[:],
                    start=True, stop=True, skip_group_check=True,
                )

            phaseL.__exit__(None, None, None)
            phaseA.__exit__(None, None, None)

            # ============== Phase B: tile offsets, counts, l_aux ==============
            totT_sb = rt.tile([E, NT], f32)
            nc.vector.tensor_copy(totT_sb[:], totT_ps[:])

            with (
                tc.tile_pool(name="psLoc", bufs=1, space="PSUM") as psLoc,
                tc.tile_pool(name="psTok", bufs=2, space="PSUM") as psTok,
            ):
                tot_ps2 = psLoc.tile([NT, E], f32, space="PSUM", tag="smallB")
                nc.tensor.transpose(
                    out=tot_ps2[:], in_=totT_sb[:], identity=ident[:E, :E]
                )
                tot_sb = rt.tile([NT, E], f32)
                nc.vector.tensor_copy(tot_sb[:], tot_ps2[:])

                offs_ps = psLoc.tile([32, E], f32, space="PSUM", tag="smallB")
                nc.tensor.matmul(
                    offs_ps[:], lhsT=u32s[:], rhs=tot_sb[:], start=True, stop=True
                )
                offs_sb = rt.tile([32, E], f32)
                nc.vector.tensor_copy(offs_sb[:], offs_ps[:])

                # own expert's per-tile offsets, replicated to all partitions
                offsT_ps = psLoc.tile([E, NT], f32, space="PSUM", tag="smallB")
                nc.tensor.transpose(
                    out=offsT_ps[:], in_=offs_sb[:], identity=ident[:NT, :NT]
                )
                offsT_sb = rt.tile([E, NT], f32)
                nc.vector.tensor_copy(offsT_sb[:], offsT_ps[:])
                ownoff_ps = psLoc.tile([1, NT], f32, space="PSUM", tag="smallB")
                nc.tensor.matmul(
                    ownoff_ps[:], lhsT=selcol[:], rhs=offsT_sb[:],
                    start=True, stop=True,
                )
                ownoff_sb = rt.tile([1, NT], f32)
                nc.vector.tensor_copy(ownoff_sb[:], ownoff_ps[:])
                offs_bc = per.tile([P, NT], f32)
                nc.gpsimd.partition_broadcast(offs_bc[:], ownoff_sb[:])

                cnt_ps = psLoc.tile([1, E], f32, space="PSUM", tag="smallB")
                nc.tensor.matmul(
                    cnt_ps[:], lhsT=onesp[:32, :], rhs=tot_sb[:],
                    start=True, stop=True,
                )
                cnt_sb = rt.tile([1, E], f32)
                nc.vector.tensor_copy(cnt_sb[:], cnt_ps[:])
                cnt_i = rt.tile([1, E], i32)
                nc.vector.tensor_copy(cnt_i[:], cnt_sb[:])
                nc.sync.dma_start(out=counts_d[:], in_=cnt_i[:])

                me_sb = rt.tile([1, E], f32)
                nc.vector.tensor_copy(me_sb[:], me_ps[:])
                lx = rt.tile([1, E], f32)
                nc.vector.tensor_tensor(
                    out=lx[:], in0=me_sb[:], in1=cnt_sb[:], op=OP.mult
                )
                lx1 = rt.tile([1, 1], f32)
                nc.vector.tensor_reduce(lx1[:], lx[:], axis=AX.X, op=OP.add)
                laux_sb = rt.tile([1, 1], f32)
                nc.scalar.mul(laux_sb[:], lx1[:], float(E) / float(T) / float(T))
                nc.sync.dma_start(out=laux_d[:], in_=laux_sb[:])

                # ===== Phase C: slots -> (token, gate) per slot via matmuls =====
                # tokmapT accumulates [token_id, gate] x slots transposed:
                # lhsT = [token,gate] pair (2-col LDW, ~free), rhs = slot
                # indicator oh (N=512 moving) -> one matmul per token tile.
                tokmapT_ps = psTok.tile([2, C], f32, space="PSUM", tag="tokmapT")
                for i4 in range(NT // 4):
                    msl = mask_all[:, i4 * 4 * E : (i4 + 1) * 4 * E]
                    loc_ps = psLoc.tile([P, 4 * E], f32, space="PSUM", tag="loc")
                    nc.tensor.matmul(
                        loc_ps[:], lhsT=u128[:], rhs=msl, start=True, stop=True
                    )
                    ownm = rt.tile([P, 4 * E], f32, tag="ownm")
                    nc.vector.tensor_tensor(
                        out=ownm[:], in0=msl, in1=sel32[:], op=OP.mult
                    )
                    own1 = rt.tile([P, 4], f32, tag="own1")
                    nc.vector.tensor_reduce(
                        own1[:], ownm[:].rearrange("p (f e) -> p f e", e=E),
                        axis=AX.X, op=OP.add,
                    )
                    lm = rt.tile([P, 4 * E], f32, tag="lm")
                    nc.vector.tensor_tensor(
                        out=lm[:], in0=loc_ps[:], in1=ownm[:], op=OP.mult
                    )
                    own_loc = rt.tile([P, 4], f32, tag="own_loc")
                    nc.vector.tensor_reduce(
                        own_loc[:], lm[:].rearrange("p (f e) -> p f e", e=E),
                        axis=AX.X, op=OP.add,
                    )
                    # global 0-based slot = tile cumsum + tile offset - 1
                    nc.vector.tensor_tensor(
                        out=own_loc[:], in0=own_loc[:],
                        in1=offs_bc[:, i4 * 4 : (i4 + 1) * 4], op=OP.add,
                    )
                    nc.vector.tensor_scalar(
                        out=own_loc[:], in0=own_loc[:], scalar1=1.0, scalar2=None,
                        op0=OP.subtract,
                    )
                    cmp = rt.tile([P, 4], f32, tag="cmp")
                    nc.vector.tensor_scalar(
                        out=cmp[:], in0=own_loc[:], scalar1=float(C), scalar2=None,
                        op0=OP.is_lt,
                    )
                    keep = rt.tile([P, 4], u32, tag="keep")
                    nc.vector.tensor_tensor(
                        out=keep[:], in0=cmp[:], in1=own1[:], op=OP.mult
                    )
                    slotf = rt.tile([P, 4], f32, tag="slotf")
                    nc.vector.select(slotf[:], keep[:], own_loc[:], bigslot4[:])

                    for j in range(4):
                        i = i4 * 4 + j
                        oh = rt.tile([P, C], f32, tag="oh")
                        nc.vector.tensor_tensor(
                            out=oh[:], in0=slotf[:, j : j + 1].to_broadcast([P, C]),
                            in1=iota512f[:], op=OP.is_equal,
                        )
                        rhs2 = rt.tile([P, 2], f32, tag="rhs2")
                        nc.vector.tensor_copy(rhs2[:, 0:1], iotatokf[:, i : i + 1])
                        nc.vector.tensor_copy(rhs2[:, 1:2], gate_all[:, i : i + 1])
                        nc.tensor.matmul(
                            tokmapT_ps[:], lhsT=rhs2[:], rhs=oh[:],
                            start=(i == 0), stop=(i == NT - 1),
                        )

                tokmapT_sb = rt.tile([2, C], f32)
                nc.vector.tensor_copy(tokmapT_sb[:], tokmapT_ps[:])
                gate_sl = per.tile([P, C // P], f32)
                tok_sl = per.tile([P, C // P], i32)
                packed_out = per.tile([P, C // P, 2], i32)
                for c in range(C // P):
                    tm_ps = psTok.tile([P, 2], f32, space="PSUM", tag="tmT")
                    nc.tensor.transpose(
                        out=tm_ps[:], in_=tokmapT_sb[:, c * P : (c + 1) * P],
                        identity=ident[:2, :2],
                    )
                    nc.vector.tensor_copy(
                        tok_sl[:, c : c + 1], tm_ps[:, 0:1]
                    )
                    nc.vector.tensor_copy(
                        gate_sl[:, c : c + 1], tm_ps[:, 1:2]
                    )
                    nc.vector.tensor_copy(
                        packed_out[:, c, 0:1], tok_sl[:, c : c + 1]
                    )
                    nc.vector.tensor_copy(
                        packed_out[:, c, 1:2], gate_sl[:, c : c + 1].bitcast(i32)
                    )

        # ================= Phase D: dispatch gather + transpose ===============
        nc.sync.dma_start(
            out=toks2_d.rearrange("(c p) two -> p c two", p=P), in_=packed_out[:]
        )
        with tc.tile_pool(name="psTr", bufs=6, space="PSUM") as psTr:
            for c in range(C // P):
                xg = dsp.tile([P, D], f32, tag="xg")
                nc.gpsimd.indirect_dma_start(
                    out=xg[:], out_offset=None,
                    in_=x_d[:],
                    in_offset=bass.IndirectOffsetOnAxis(ap=tok_sl[:, c : c + 1], axis=0),
                    bounds_check=T - 1, oob_is_err=False,
                )
                for k in range(DK):
                    tp = psTr.tile([P, P], f32, space="PSUM", tag="tpD")
                    nc.tensor.transpose(
                        out=tp[:], in_=xg[:, k * P : (k + 1) * P], identity=ident[:]
                    )
                    nc.vector.tensor_copy(
                        xte[:, k, c * P : (c + 1) * P], tp[:]
                    )

        # ================= Phase E: GEMM1 + gelu -> hT ========================
        with tc.tile_pool(name="psH", bufs=6, space="PSUM") as psH:
            for g in range(8):          # groups of 4 DFF tiles
                w1g = w1p.tile([P, DK, 512], f32r, tag="w1g")
                for k in range(DK):
                    nc.sync.dma_start(
                        out=w1g[:, k, :],
                        in_=w1_d[k * P : (k + 1) * P, g * 512 : (g + 1) * 512].bitcast(f32r),
                    )
                for mm in range(4):
                    mt = g * 4 + mm
                    h_ps = psH.tile([P, C], f32, space="PSUM", tag="h")
                    for k in range(DK):
                        nc.tensor.matmul(
                            h_ps[:],
                            lhsT=w1g[:, k, mm * P : (mm + 1) * P],
                            rhs=xte[:, k, :],
                            start=(k == 0),
                            stop=(k == DK - 1),
                        )
                    nc.scalar.activation(
                        ht[:, mt, :], h_ps[:], ACTF.Gelu_apprx_tanh,
                        bias=b1c[:, mt : mt + 1],
                    )

        # ================= Phase F: GEMM2 + bias + gate scale =================
        with tc.tile_pool(name="psO", bufs=1, space="PSUM") as psO:
            o_ps = []
            for ct in range(4):
                row = []
                for n in range(2):
                    o_tile = psO.tile([P, 512], f32, space="PSUM", tag=f"o{ct}{n}",
                                      name=f"o_ps_{ct}_{n}")
                    row.append(o_tile)
                o_ps.append(row)
            for k in range(FK):
                w2s = w2p.tile([P, D], f32r, tag="w2s")
                nc.sync.dma_start(
                    out=w2s[:], in_=w2_d[k * P : (k + 1) * P, :].bitcast(f32r))
                for ct in range(4):
                    for n in range(2):
                        nc.tensor.matmul(
                            o_ps[ct][n][:],
                            lhsT=ht[:, k, ct * P : (ct + 1) * P],
                            rhs=w2s[:, n * 512 : (n + 1) * 512],
                            start=(k == 0),
                            stop=(k == FK - 1),
                        )
            for ct in range(4):
                for n in range(2):
                    ot = outp.tile([P, 512], f32, tag="ot")
                    nc.vector.tensor_tensor(
                        out=ot[:], in0=o_ps[ct][n][:],
                        in1=b2rep[:, n * 512 : (n + 1) * 512], op=OP.add,
                    )
                    nc.scalar.mul(ot[:], ot[:], gate_sl[:, ct : ct + 1])
                    nc.sync.dma_start(
                        out=rows_d[ct * P : (ct + 1) * P, n * 512 : (n + 1) * 512],
                        in_=ot[:],
                    )


# ---------------------------------------------------------------------------
# Host-side: build/compile once, shard, run SPMD, unshard.
# ---------------------------------------------------------------------------

_CACHE = {}


def _build():
    import concourse.bacc as bacc
    import concourse.mybir as mybir
    import concourse.tile as tile

    f32 = mybir.dt.float32
    i32 = mybir.dt.int32

    nc = bacc.Bacc("TRN2", target_bir_lowering=False, debug=False,
                   num_devices=N_CORES)

    ins = {
        "x": nc.dram_tensor("x", [T, D], f32, kind="ExternalInput"),
        "wgr": nc.dram_tensor("wgr", [P, DK * E], f32, kind="ExternalInput"),
        "w1": nc.dram_tensor("w1", [D, DFF], mybir.dt.float32r, kind="ExternalInput"),
        "b1c": nc.dram_tensor("b1c", [P, FK], f32, kind="ExternalInput"),
        "w2": nc.dram_tensor("w2", [DFF, D], mybir.dt.float32r, kind="ExternalInput"),
        "b2rep": nc.dram_tensor("b2rep", [P, D], f32, kind="ExternalInput"),
        "selrep": nc.dram_tensor("selrep", [P, E], f32, kind="ExternalInput"),
        "ident": nc.dram_tensor("ident", [P, P], f32, kind="ExternalInput"),
        "u128": nc.dram_tensor("u128", [P, P], f32, kind="ExternalInput"),
        "u32s": nc.dram_tensor("u32s", [32, 32], f32, kind="ExternalInput"),
        "onesp": nc.dram_tensor("onesp", [P, 1], f32, kind="ExternalInput"),
        "selcol": nc.dram_tensor("selcol", [E, 1], f32, kind="ExternalInput"),
    }
    outs = {
        "rows": nc.dram_tensor("rows", [C, D], f32, kind="ExternalOutput"),
        "toks2": nc.dram_tensor("toks2", [C, 2], i32, kind="ExternalOutput"),
        "counts": nc.dram_tensor("counts", [1, E], i32, kind="ExternalOutput"),
        "laux": nc.dram_tensor("laux", [1, 1], f32, kind="ExternalOutput"),
    }
    with tile.TileContext(nc) as tc:
        emit_moe(tc, {k: v.ap() for k, v in outs.items()},
                 {k: v.ap() for k, v in ins.items()})
    nc.compile()
    return nc


def host_inputs(hidden_states, wg, w1, b1, w2, b2):
    """Per-core input maps (host-side shard/layout prep only)."""
    x = np.ascontiguousarray(np.asarray(hidden_states, np.float32).reshape(T, D))
    wg = np.asarray(wg, np.float32)
    w1 = np.asarray(w1, np.float32)
    b1 = np.asarray(b1, np.float32)
    w2 = np.asarray(w2, np.float32)
    b2 = np.asarray(b2, np.float32)

    # wg rearranged k-major: wgr[p, k*E+e] = wg[k*128+p, e]
    wgr = np.ascontiguousarray(
        wg.reshape(DK, P, E).transpose(1, 0, 2).reshape(P, DK * E)
    )
    ident = np.eye(P, dtype=np.float32)
    u128 = np.triu(np.ones((P, P), np.float32))        # incl diag
    u32s = np.triu(np.ones((32, 32), np.float32), 1)   # strictly upper
    onesp = np.ones((P, 1), np.float32)

    in_maps = []
    for e in range(N_CORES):
        b1c = np.ascontiguousarray(b1[e].reshape(FK, P).T)       # [128, FK]
        b2rep = np.ascontiguousarray(np.tile(b2[e][None, :], (P, 1)))
        selrep = np.zeros((P, E), np.float32)
        selrep[:, e] = 1.0
        selcol = np.zeros((E, 1), np.float32)
        selcol[e, 0] = 1.0
        in_maps.append({
            "x": x, "wgr": wgr,
            "w1": np.ascontiguousarray(w1[e]),
            "b1c": b1c,
            "w2": np.ascontiguousarray(w2[e]),
            "b2rep": b2rep, "selrep": selrep,
            "ident": ident, "u128": u128, "u32s": u32s,
            "onesp": onesp, "selcol": selcol,
        })
    return in_maps


def combine(results):
    """Host unshard: scatter each expert's kept rows back to token slots."""
    counts = results[0]["counts"][0].astype(np.int64)
    out = np.zeros((T, D), np.float32)
    for e in range(N_CORES):
        k = int(min(counts[e], C))
        if k > 0:
            tok = results[e]["toks2"][:k, 0].astype(np.int64)
            out[tok] = results[e]["rows"][:k]
    l_aux = np.float32(results[0]["laux"][0, 0])
    exp_counts = results[0]["counts"][0].astype(np.int32)
    return out.reshape(B, S, D), l_aux, exp_counts


def kernel(hidden_states, wg, w1, b1, w2, b2):
    from concourse.bass_utils import run_bass_kernel_spmd

    if "nc" not in _CACHE:
        _CACHE["nc"] = _build()
    nc = _CACHE["nc"]
    in_maps = host_inputs(hidden_states, wg, w1, b1, w2, b2)
    res = run_bass_kernel_spmd(nc, in_maps, list(range(N_CORES)))
    return combine(res.results)
